# revision 1
# baseline (speedup 1.0000x reference)
"""GBST embedding kernel for Trainium2, data-parallel over batch on 8 cores.

Strategy per core (one batch element, [d_chunk, l] layout, 4 chunks of 128):
- Embedding gather is folded into the conv: y[do,l] = sum_k sum_v G_k[v,do] *
  onehot[v, l+k-2] with G_k = emb @ conv_w[:,:,k].T precomputed on host (bf16).
  Contraction over vocab (256 = 2 chunks) instead of d_in (512) halves PE work.
- Onehot built on device: ids broadcast via replicate-DMA + is_equal vs iota.
- Scores: s1 = score_w.T @ y on PE; block-pool sums for b=2,3,4 as strided adds.
- Softmax over the 4 upsampled scores in an l-major [128, 64] layout; softmax
  weights folded with 0.5/b (mean-pool scale + final downsample-by-2 scale) and
  collapsed onto the output t-grid (b=2,4 fully; b=3 onto a u-grid of 3-blocks).
- Weighted combine on DVE with bf16 muls and f32 accumulation; block pooling
  recomputed per segment to keep SBUF small; weight rows staged via DRAM and
  broadcast back with replicate DMAs. PE transposes [d, t] -> [t, d] for
  contiguous stores.
"""
import sys
sys.path.insert(0, "/opt/trn_rl_repo")
import numpy as np
import ml_dtypes

import concourse.bass as bass
import concourse.bacc as bacc
import concourse.tile as tile
from concourse import mybir
from concourse.bass_utils import run_bass_kernel_spmd

bf16 = ml_dtypes.bfloat16
F32 = mybir.dt.float32
BF = mybir.dt.bfloat16
OP = mybir.AluOpType

L, T, V, D, K = 8192, 4096, 256, 512, 5
NDC, NVC, NLT, LTS = 4, 2, 16, 512
N3 = 2731          # ceil(L/3)
TSEG = 1024        # combine segment width in t

TRACE = False
LAST_RESULT = None
_NC = None


def _build():
    nc = bacc.Bacc("TRN2", target_bir_lowering=False)
    ids_d = nc.dram_tensor("ids", [1, L], BF, kind="ExternalInput")
    gws_d = nc.dram_tensor("gws", [128, 40 * 128], BF, kind="ExternalInput")
    iot_d = nc.dram_tensor("iot", [128, 2], F32, kind="ExternalInput")
    scw_d = nc.dram_tensor("scw", [128, 4], BF, kind="ExternalInput")
    bias_d = nc.dram_tensor("bias", [128, 4], F32, kind="ExternalInput")
    ident_d = nc.dram_tensor("ident", [128, 128], F32, kind="ExternalInput")
    out_d = nc.dram_tensor("out", [T, D], F32, kind="ExternalOutput")
    # DRAM staging for broadcast-source weight rows
    w1erow_d = nc.dram_tensor("w1erow_d", [1, T], BF)
    w1orow_d = nc.dram_tensor("w1orow_d", [1, T], BF)
    cw2row_d = nc.dram_tensor("cw2row_d", [1, T], BF)
    cw4row_d = nc.dram_tensor("cw4row_d", [1, T], BF)
    cwarow_d = nc.dram_tensor("cwarow_d", [1, 1366], BF)
    cwbrow_d = nc.dram_tensor("cwbrow_d", [1, 1366], BF)
    cwcrow_d = nc.dram_tensor("cwcrow_d", [1, 1366], BF)
    cwdrow_d = nc.dram_tensor("cwdrow_d", [1, 1366], BF)

    with tile.TileContext(nc) as tc:
        with tc.tile_pool(name="const", bufs=1) as cst, \
             tc.tile_pool(name="persist", bufs=1) as per, \
             tc.tile_pool(name="rows", bufs=1) as rws, \
             tc.tile_pool(name="rowbig", bufs=1) as rwb, \
             tc.tile_pool(name="sm", bufs=1) as sm, \
             tc.tile_pool(name="ohp", bufs=2) as ohp, \
             tc.tile_pool(name="wseg", bufs=1) as wsg, \
             tc.tile_pool(name="segpool", bufs=2) as sgp, \
             tc.tile_pool(name="accp", bufs=1) as accp, \
             tc.tile_pool(name="ctp", bufs=3) as ctp, \
             tc.tile_pool(name="btp", bufs=2) as btp, \
             tc.tile_pool(name="otp", bufs=2) as otp, \
             tc.tile_pool(name="psA", bufs=3, space="PSUM") as psA, \
             tc.tile_pool(name="psB", bufs=2, space="PSUM") as psB, \
             tc.tile_pool(name="psT", bufs=2, space="PSUM") as psT:

            # ---- constants
            gws_t = cst.tile([128, 40 * 128], BF)
            nc.sync.dma_start(out=gws_t[:], in_=gws_d[:])
            iot_t = cst.tile([128, 2], F32)
            nc.sync.dma_start(out=iot_t[:], in_=iot_d[:])
            scw_t = cst.tile([128, 4], BF)
            nc.sync.dma_start(out=scw_t[:], in_=scw_d[:])
            bias_t = cst.tile([128, 4], F32)
            nc.sync.dma_start(out=bias_t[:], in_=bias_d[:])
            ident_t = cst.tile([128, 128], F32)
            nc.sync.dma_start(out=ident_t[:], in_=ident_d[:])

            # ---- persistent tensors
            y = [per.tile([128, L + 4], BF, name=f"y{dc}", tag=f"y{dc}")
                 for dc in range(NDC)]
            s1row = rws.tile([1, L + 4], F32)
            s3row = rws.tile([1, N3], F32)
            cwarow = rws.tile([1, 1366], BF)
            cwbrow = rws.tile([1, 1366], BF)
            cwcrow = rws.tile([1, 1366], BF)
            cwdrow = rws.tile([1, 1366], BF)

            for dc in range(NDC):
                nc.vector.memset(y[dc][:, L:L + 4], 0.0)
            nc.vector.memset(s1row[0:1, L:L + 4], 0.0)

            # ---- conv + gather + s1, per l-tile
            for i in range(NLT):
                c0 = i * LTS - 2
                c1 = i * LTS + 514
                lo = max(c0, 0)
                hi = min(c1, L)
                d0 = lo - c0          # dst col where valid data starts
                d1 = 516 - (c1 - hi)  # dst col where valid data ends
                idst = ohp.tile([128, 516], BF, tag="idst")
                nc.sync.dma_start(out=idst[:, d0:d1],
                                  in_=ids_d[0:1, lo:hi].partition_broadcast(128))
                ohs = []
                for vc in range(NVC):
                    oh = ohp.tile([128, 516], BF, tag=f"oh{vc}", name=f"oh{vc}_{i}")
                    if d0 > 0:
                        nc.vector.memset(oh[:, 0:d0], 0.0)
                    if d1 < 516:
                        nc.vector.memset(oh[:, d1:516], 0.0)
                    nc.vector.tensor_scalar(out=oh[:, d0:d1], in0=idst[:, d0:d1],
                                            scalar1=iot_t[:, vc:vc + 1], scalar2=None,
                                            op0=OP.is_equal)
                    ohs.append(oh)
                for dc in range(NDC):
                    ps = psA.tile([128, LTS], F32, tag="convps", name=f"ps_{i}_{dc}")
                    for j in range(10):
                        k, vc = divmod(j, 2)
                        nc.tensor.matmul(
                            out=ps[:],
                            lhsT=gws_t[:, ((k * 2 + vc) * 4 + dc) * 128:
                                       ((k * 2 + vc) * 4 + dc) * 128 + 128],
                            rhs=ohs[vc][:, k:k + LTS],
                            start=(j == 0), stop=(j == 9))
                    nc.scalar.activation(out=y[dc][:, i * LTS:(i + 1) * LTS], in_=ps[:],
                                         func=mybir.ActivationFunctionType.Identity,
                                         bias=bias_t[:, dc:dc + 1])
                ps1 = psB.tile([1, LTS], F32, tag="s1ps", name=f"ps1_{i}")
                for dc in range(NDC):
                    nc.tensor.matmul(out=ps1[:], lhsT=scw_t[:, dc:dc + 1],
                                     rhs=y[dc][:, i * LTS:(i + 1) * LTS],
                                     start=(dc == 0), stop=(dc == NDC - 1))
                nc.scalar.copy(out=s1row[0:1, i * LTS:(i + 1) * LTS], in_=ps1[:])

            # ---- score pooling + softmax in l-major [128, 64] layout
            S = sm.tile([128, 256], F32)
            nc.sync.dma_start(out=S[:, 0:64], in_=s1row[0:1, 0:L])
            s2r = sm.tile([128, 32], F32)
            Spair = S[:, 0:64].rearrange("p (n two) -> p n two", two=2)
            nc.vector.tensor_tensor(out=s2r[:], in0=Spair[:, :, 0],
                                    in1=Spair[:, :, 1], op=OP.add)
            s4r = sm.tile([128, 16], F32)
            s2pair = s2r[:].rearrange("p (n two) -> p n two", two=2)
            nc.vector.tensor_tensor(out=s4r[:], in0=s2pair[:, :, 0],
                                    in1=s2pair[:, :, 1], op=OP.add)
            nc.vector.tensor_scalar(
                out=S[:, 64:128].rearrange("p (n two) -> p n two", two=2),
                in0=s2r[:].unsqueeze(2).to_broadcast([128, 32, 2]),
                scalar1=0.5, scalar2=None, op0=OP.mult)
            nc.vector.tensor_scalar(
                out=S[:, 192:256].rearrange("p (n four) -> p n four", four=4),
                in0=s4r[:].unsqueeze(2).to_broadcast([128, 16, 4]),
                scalar1=0.25, scalar2=None, op0=OP.mult)
            nc.vector.tensor_tensor(out=s3row[0:1, :], in0=s1row[0:1, 0:3 * N3:3],
                                    in1=s1row[0:1, 1:3 * N3 + 1:3], op=OP.add)
            nc.vector.tensor_tensor(out=s3row[0:1, :], in0=s3row[0:1, :],
                                    in1=s1row[0:1, 2:3 * N3 + 2:3], op=OP.add)
            us3row = rwb.tile([1, 3 * N3], F32, tag="rowbig")
            nc.vector.tensor_copy(
                out=us3row[0:1, :],
                in_=s3row[0:1, :].unsqueeze(2).to_broadcast([1, N3, 3]))
            nc.sync.dma_start(out=S[:, 128:192], in_=us3row[0:1, 0:L])
            nc.vector.tensor_scalar(out=S[:, 128:192], in0=S[:, 128:192],
                                    scalar1=1.0 / 3.0, scalar2=None, op0=OP.mult)

            mM = sm.tile([128, 64], F32)
            nc.vector.tensor_tensor(out=mM[:], in0=S[:, 0:64], in1=S[:, 64:128],
                                    op=OP.max)
            nc.vector.tensor_tensor(out=mM[:], in0=mM[:], in1=S[:, 128:192], op=OP.max)
            nc.vector.tensor_tensor(out=mM[:], in0=mM[:], in1=S[:, 192:256], op=OP.max)
            S4v = S[:].rearrange("p (four n) -> p four n", four=4)
            nc.vector.tensor_tensor(out=S4v, in0=S4v,
                                    in1=mM[:].unsqueeze(1).to_broadcast([128, 4, 64]),
                                    op=OP.subtract)
            nc.scalar.activation(out=S[:], in_=S[:],
                                 func=mybir.ActivationFunctionType.Exp)
            Z = sm.tile([128, 64], F32)
            nc.vector.tensor_tensor(out=Z[:], in0=S[:, 0:64], in1=S[:, 64:128],
                                    op=OP.add)
            nc.vector.tensor_tensor(out=Z[:], in0=Z[:], in1=S[:, 128:192], op=OP.add)
            nc.vector.tensor_tensor(out=Z[:], in0=Z[:], in1=S[:, 192:256], op=OP.add)
            R = sm.tile([128, 64], F32)
            nc.vector.reciprocal(out=R[:], in_=Z[:])
            W = sm.tile([128, 256], F32)
            W4v = W[:].rearrange("p (four n) -> p four n", four=4)
            nc.vector.tensor_tensor(out=W4v, in0=S4v,
                                    in1=R[:].unsqueeze(1).to_broadcast([128, 4, 64]),
                                    op=OP.mult)
            # weight extraction, 0.5/b folded
            W1e = sm.tile([128, 32], BF)
            W1o = sm.tile([128, 32], BF)
            W1pair = W[:, 0:64].rearrange("p (n two) -> p n two", two=2)
            nc.vector.tensor_scalar(out=W1e[:], in0=W1pair[:, :, 0], scalar1=0.5,
                                    scalar2=None, op0=OP.mult)
            nc.vector.tensor_scalar(out=W1o[:], in0=W1pair[:, :, 1], scalar1=0.5,
                                    scalar2=None, op0=OP.mult)
            tmp32 = sm.tile([128, 32], F32)
            W2pair = W[:, 64:128].rearrange("p (n two) -> p n two", two=2)
            nc.vector.tensor_tensor(out=tmp32[:], in0=W2pair[:, :, 0],
                                    in1=W2pair[:, :, 1], op=OP.add)
            CW2 = sm.tile([128, 32], BF)
            nc.vector.tensor_scalar(out=CW2[:], in0=tmp32[:], scalar1=0.25,
                                    scalar2=None, op0=OP.mult)
            tmp32b = sm.tile([128, 32], F32)
            W4pair = W[:, 192:256].rearrange("p (n two) -> p n two", two=2)
            nc.vector.tensor_tensor(out=tmp32b[:], in0=W4pair[:, :, 0],
                                    in1=W4pair[:, :, 1], op=OP.add)
            CW4 = sm.tile([128, 32], BF)
            nc.vector.tensor_scalar(out=CW4[:], in0=tmp32b[:], scalar1=0.125,
                                    scalar2=None, op0=OP.mult)
            W3 = sm.tile([128, 64], BF)
            nc.vector.tensor_scalar(out=W3[:], in0=W[:, 128:192], scalar1=1.0 / 6.0,
                                    scalar2=None, op0=OP.mult)
            # rows: reshape DMAs to DRAM staging; b3 u-grid rows via w3row
            nc.sync.dma_start(out=w1erow_d[0:1, :], in_=W1e[:])
            nc.sync.dma_start(out=w1orow_d[0:1, :], in_=W1o[:])
            nc.sync.dma_start(out=cw2row_d[0:1, :], in_=CW2[:])
            nc.sync.dma_start(out=cw4row_d[0:1, :], in_=CW4[:])
            w3row = rwb.tile([1, L + 10], BF, tag="rowbig")
            nc.vector.memset(w3row[0:1, L:L + 10], 0.0)
            nc.sync.dma_start(out=w3row[0:1, 0:L], in_=W3[:])
            nc.vector.tensor_tensor(out=cwarow[0:1, :], in0=w3row[0:1, 0:8196:6],
                                    in1=w3row[0:1, 1:8197:6], op=OP.add)
            nc.vector.tensor_copy(out=cwbrow[0:1, :], in_=w3row[0:1, 2:8198:6])
            nc.vector.tensor_copy(out=cwcrow[0:1, :], in_=w3row[0:1, 3:8199:6])
            nc.vector.tensor_tensor(out=cwdrow[0:1, :], in0=w3row[0:1, 4:8200:6],
                                    in1=w3row[0:1, 5:8201:6], op=OP.add)
            nc.sync.dma_start(out=cwarow_d[:], in_=cwarow[:])
            nc.sync.dma_start(out=cwbrow_d[:], in_=cwbrow[:])
            nc.sync.dma_start(out=cwcrow_d[:], in_=cwcrow[:])
            nc.sync.dma_start(out=cwdrow_d[:], in_=cwdrow[:])

            # ---- combine + transpose + store, segmented over t
            ov = out_d[:].rearrange("(tb p) (dc c) -> p tb dc c", p=128, c=128)
            for s in range(T // TSEG):
                t0 = s * TSEG
                # u-grid windows for the three b=3 residue classes
                tA0 = t0 + (-t0) % 3
                nA = len(range(tA0, t0 + TSEG, 3))
                uA0 = tA0 // 3
                tB0 = t0 + (1 - t0) % 3
                nB = len(range(tB0, t0 + TSEG, 3))
                uB0 = (tB0 - 1) // 3
                tD0 = t0 + (2 - t0) % 3
                nD = len(range(tD0, t0 + TSEG, 3))
                uD0 = (tD0 - 2) // 3
                jbase = min(2 * uA0, 2 * uB0, 2 * uD0 + 1)
                jend = max(2 * (uA0 + nA - 1), 2 * (uB0 + nB - 1) + 1,
                           2 * (uD0 + nD - 1) + 1)
                nJ = jend - jbase + 1

                w1e_s = wsg.tile([128, TSEG], BF, tag="w1e", name=f"w1e_{s}")
                nc.sync.dma_start(
                    out=w1e_s[:],
                    in_=w1erow_d[0:1, t0:t0 + TSEG].partition_broadcast(128))
                w1o_s = wsg.tile([128, TSEG], BF, tag="w1o", name=f"w1o_{s}")
                nc.sync.dma_start(
                    out=w1o_s[:],
                    in_=w1orow_d[0:1, t0:t0 + TSEG].partition_broadcast(128))
                cw2_s = wsg.tile([128, TSEG], BF, tag="cw2", name=f"cw2_{s}")
                nc.sync.dma_start(
                    out=cw2_s[:],
                    in_=cw2row_d[0:1, t0:t0 + TSEG].partition_broadcast(128))
                cw4_s = wsg.tile([128, TSEG], BF, tag="cw4", name=f"cw4_{s}")
                nc.sync.dma_start(
                    out=cw4_s[:],
                    in_=cw4row_d[0:1, t0:t0 + TSEG].partition_broadcast(128))
                cwa_s = wsg.tile([128, 342], BF, tag="cwa", name=f"cwa_{s}")
                nc.sync.dma_start(
                    out=cwa_s[:, 0:nA],
                    in_=cwarow_d[0:1, uA0:uA0 + nA].partition_broadcast(128))
                cwb_s = wsg.tile([128, 342], BF, tag="cwb", name=f"cwb_{s}")
                nc.sync.dma_start(
                    out=cwb_s[:, 0:nB],
                    in_=cwbrow_d[0:1, uB0:uB0 + nB].partition_broadcast(128))
                cwc_s = wsg.tile([128, 342], BF, tag="cwc", name=f"cwc_{s}")
                nc.sync.dma_start(
                    out=cwc_s[:, 0:nB],
                    in_=cwcrow_d[0:1, uB0:uB0 + nB].partition_broadcast(128))
                cwd_s = wsg.tile([128, 342], BF, tag="cwd", name=f"cwd_{s}")
                nc.sync.dma_start(
                    out=cwd_s[:, 0:nD],
                    in_=cwdrow_d[0:1, uD0:uD0 + nD].partition_broadcast(128))

                for dc in range(NDC):
                    ypair = y[dc][:, 0:L].rearrange("p (t two) -> p t two", two=2)
                    # per-seg pooled blocks
                    p2_s = sgp.tile([128, TSEG], BF, tag="p2s", name=f"p2s_{s}_{dc}")
                    nc.vector.tensor_tensor(out=p2_s[:], in0=ypair[:, t0:t0 + TSEG, 0],
                                            in1=ypair[:, t0:t0 + TSEG, 1], op=OP.add)
                    p4_s = sgp.tile([128, TSEG // 2], BF, tag="p4s",
                                    name=f"p4s_{s}_{dc}")
                    p2sp = p2_s[:].rearrange("p (v two) -> p v two", two=2)
                    nc.vector.tensor_tensor(out=p4_s[:], in0=p2sp[:, :, 0],
                                            in1=p2sp[:, :, 1], op=OP.add)
                    p3_s = sgp.tile([128, 688], BF, tag="p3s", name=f"p3s_{s}_{dc}")
                    nc.vector.tensor_tensor(
                        out=p3_s[:, 0:nJ],
                        in0=y[dc][:, 3 * jbase:3 * (jbase + nJ) - 2:3],
                        in1=y[dc][:, 3 * jbase + 1:3 * (jbase + nJ) - 1:3], op=OP.add)
                    nc.vector.tensor_tensor(
                        out=p3_s[:, 0:nJ], in0=p3_s[:, 0:nJ],
                        in1=y[dc][:, 3 * jbase + 2:3 * (jbase + nJ):3], op=OP.add)

                    m1e = ctp.tile([128, TSEG], BF, tag="ct", name=f"m1e_{s}_{dc}")
                    nc.vector.tensor_tensor(out=m1e[:], in0=ypair[:, t0:t0 + TSEG, 0],
                                            in1=w1e_s[:], op=OP.mult)
                    m1o = ctp.tile([128, TSEG], BF, tag="ct", name=f"m1o_{s}_{dc}")
                    nc.vector.tensor_tensor(out=m1o[:], in0=ypair[:, t0:t0 + TSEG, 1],
                                            in1=w1o_s[:], op=OP.mult)
                    a12 = ctp.tile([128, TSEG], BF, tag="ct", name=f"a12_{s}_{dc}")
                    nc.vector.tensor_tensor(out=a12[:], in0=m1e[:], in1=m1o[:],
                                            op=OP.add)
                    m2 = ctp.tile([128, TSEG], BF, tag="ct", name=f"m2_{s}_{dc}")
                    nc.vector.tensor_tensor(out=m2[:], in0=p2_s[:],
                                            in1=cw2_s[:], op=OP.mult)
                    m4 = ctp.tile([128, TSEG], BF, tag="ct", name=f"m4_{s}_{dc}")
                    nc.vector.tensor_tensor(
                        out=m4[:],
                        in0=p4_s[:].unsqueeze(2).to_broadcast([128, TSEG // 2, 2]),
                        in1=cw4_s[:], op=OP.mult)
                    acc = accp.tile([128, TSEG], F32, tag="acc", name=f"acc_{s}_{dc}")
                    nc.vector.tensor_tensor(out=acc[:], in0=a12[:], in1=m2[:],
                                            op=OP.add)
                    nc.vector.tensor_tensor(out=acc[:], in0=acc[:], in1=m4[:],
                                            op=OP.add)
                    # b=3 terms on their u-grids
                    for bi, (n_u, u0, coff, poff, cw_s) in enumerate((
                            (nA, uA0, tA0 - t0, 0, cwa_s),
                            (nB, uB0, tB0 - t0, 0, cwb_s),
                            (nB, uB0, tB0 - t0, 1, cwc_s),
                            (nD, uD0, tD0 - t0, 1, cwd_s))):
                        tb3 = btp.tile([128, 342], BF, tag="bt",
                                       name=f"tb3_{s}_{dc}_{bi}")
                        j0 = 2 * u0 + poff - jbase
                        nc.vector.tensor_tensor(
                            out=tb3[:, 0:n_u],
                            in0=p3_s[:, j0:j0 + 2 * n_u - 1:2],
                            in1=cw_s[:, 0:n_u], op=OP.mult)
                        accv = acc[:, coff:TSEG:3]
                        nc.vector.tensor_tensor(out=accv[:, 0:n_u], in0=accv[:, 0:n_u],
                                                in1=tb3[:, 0:n_u], op=OP.add)
                    # transpose [d, t] -> [t, d] and store
                    for q4 in range(2):
                        pt = psT.tile([128, 512], F32, tag="tp",
                                      name=f"pt_{s}_{dc}_{q4}")
                        for q in range(4):
                            nc.tensor.transpose(
                                out=pt[:, q * 128:(q + 1) * 128],
                                in_=acc[:, (q4 * 4 + q) * 128:(q4 * 4 + q + 1) * 128],
                                identity=ident_t[:])
                        ot = otp.tile([128, 512], F32, tag="ot",
                                      name=f"ot_{s}_{dc}_{q4}")
                        nc.scalar.copy(out=ot[:], in_=pt[:])
                        tb0 = s * 8 + q4 * 4
                        nc.sync.dma_start(
                            out=ov[:, tb0:tb0 + 4, dc, :],
                            in_=ot[:].rearrange("p (tb c) -> p tb c", c=128))
    nc.compile()
    return nc


def _get_nc():
    global _NC
    if _NC is None:
        _NC = _build()
    return _NC


def kernel(input_ids, emb, conv_w, conv_b, score_w):
    global LAST_RESULT
    nc = _get_nc()
    input_ids = np.asarray(input_ids)
    emb = np.asarray(emb, dtype=np.float32)
    conv_w = np.asarray(conv_w, dtype=np.float32)
    conv_b = np.asarray(conv_b, dtype=np.float32)
    score_w = np.asarray(score_w, dtype=np.float32)
    B = input_ids.shape[0]

    G = np.einsum("oik,vi->kvo", conv_w.astype(np.float64),
                  emb.astype(np.float64)).astype(np.float32)  # [K, V, D]
    gws = np.zeros((128, 40, 128), np.float32)
    for k in range(K):
        for vc in range(NVC):
            for dc in range(NDC):
                gws[:, (k * 2 + vc) * 4 + dc, :] = \
                    G[k, vc * 128:(vc + 1) * 128, dc * 128:(dc + 1) * 128]
    gws = gws.reshape(128, 40 * 128).astype(bf16)
    iot = np.stack([np.arange(128), np.arange(128) + 128], axis=1).astype(np.float32)
    scw = score_w.reshape(4, 128).T.astype(bf16)
    biasm = conv_b.reshape(4, 128).T.astype(np.float32)
    ident = np.eye(128, dtype=np.float32)
    idsb = input_ids.astype(np.float32).astype(bf16)

    in_maps = [{"ids": np.ascontiguousarray(idsb[c:c + 1]), "gws": gws, "iot": iot,
                "scw": scw, "bias": biasm, "ident": ident} for c in range(B)]
    res = run_bass_kernel_spmd(nc, in_maps, core_ids=list(range(B)), trace=TRACE)
    LAST_RESULT = res
    return np.stack([res.results[c]["out"] for c in range(B)]).astype(np.float32)



# revision 2
# speedup vs baseline: 1.4103x; 1.4103x over previous
"""GBST embedding kernel for Trainium2, data-parallel over batch on 8 cores.

v2: pipelined per-segment structure.
- Conv folded with embedding gather via onehot matmul (vocab contraction),
  y stored in even/odd deinterleaved planes (unit-stride DVE combine).
- b2+b4 combine terms folded into per-t row weights we/wo (one product per
  y plane); b4 residue handled via a reversed-pair strided view of p2.
- b3 handled on the u-grid with 4 residue-class rows, products unit-stride.
- Softmax per 2048-l segment in l-major [128,16] layout; s3 pooling done
  multi-partition via a [114,18] reshape trick.
- Output written bf16 via DMA xbar transpose (no PE transposes); host
  converts to f32.
"""
import sys
sys.path.insert(0, "/opt/trn_rl_repo")
import numpy as np
import ml_dtypes

import concourse.bass as bass
import concourse.bacc as bacc
import concourse.tile as tile
from concourse import mybir
from concourse.bass_utils import run_bass_kernel_spmd

bf16 = ml_dtypes.bfloat16
F32 = mybir.dt.float32
BF = mybir.dt.bfloat16
OP = mybir.AluOpType

L, T, V, D, K = 8192, 4096, 256, 512, 5
NDC, NVC, NLT, LTS = 4, 2, 16, 512
NSEG, SEGT, SEGL = 4, 1024, 2048
HP = 4104          # per-parity plane width (4096 + 8 pad)

TRACE = False
LAST_RESULT = None
_NC = None


def _seg_windows(s):
    """Per-seg b3 class windows on the m-grid (t=3m/3m+1/3m+2)."""
    t0, t1 = s * SEGT, (s + 1) * SEGT
    mA0 = -(-t0 // 3)
    nA = len(range(3 * mA0, t1, 3))
    mB0 = -(-(t0 - 1) // 3)
    nB = len(range(3 * mB0 + 1, t1, 3))
    mD0 = -(-(t0 - 2) // 3)
    nD = len(range(3 * mD0 + 2, t1, 3))
    mJ0 = min(mA0, mB0, mD0)
    mHI = max(mA0 + nA, mB0 + nB, mD0 + nD)
    return t0, mA0, nA, mB0, nB, mD0, nD, mJ0, mHI




def _sl(ap, start, n, step):
    """Tight strided slice of n elements from an AP's free dim."""
    return ap[:, start:start + step * (n - 1) + 1:step]


def _sl1(ap, start, n, step):
    return ap[0:1, start:start + step * (n - 1) + 1:step]


def _build():
    nc = bacc.Bacc("TRN2", target_bir_lowering=False)
    ids_d = nc.dram_tensor("ids", [1, L], BF, kind="ExternalInput")
    gws_d = nc.dram_tensor("gws", [128, 40 * 128], BF, kind="ExternalInput")
    iot_d = nc.dram_tensor("iot", [128, 2], F32, kind="ExternalInput")
    scw_d = nc.dram_tensor("scw", [128, 4], BF, kind="ExternalInput")
    bias_d = nc.dram_tensor("bias", [128, 4], F32, kind="ExternalInput")
    out_d = nc.dram_tensor("out", [T, D], BF, kind="ExternalOutput")
    # DRAM staging
    s1row_d = nc.dram_tensor("s1row_d", [1, L + 16], F32)
    us3_d = nc.dram_tensor("us3_d", [1, NSEG * 2052], F32)
    w3row_d = nc.dram_tensor("w3row_d", [1, L + 16], BF)
    we_d = nc.dram_tensor("we_d", [1, T], BF)
    wo_d = nc.dram_tensor("wo_d", [1, T], BF)
    cw4_d = nc.dram_tensor("cw4_d", [1, T], BF)
    cwrow_d = [nc.dram_tensor(f"cw{cl}_d", [1, NSEG * 348], BF)
               for cl in "abcd"]

    with tile.TileContext(nc) as tc:
        with tc.tile_pool(name="const", bufs=1) as cst, \
             tc.tile_pool(name="ydp", bufs=1) as ydp, \
             tc.tile_pool(name="ohp", bufs=2) as ohp, \
             tc.tile_pool(name="s1p", bufs=2) as s1p, \
             tc.tile_pool(name="smp", bufs=2) as smp, \
             tc.tile_pool(name="rwp", bufs=2) as rwp, \
             tc.tile_pool(name="wsg", bufs=2) as wsg, \
             tc.tile_pool(name="cmb", bufs=2) as cmb, \
             tc.tile_pool(name="ctp", bufs=3) as ctp, \
             tc.tile_pool(name="otp", bufs=3) as otp, \
             tc.tile_pool(name="psA", bufs=3, space="PSUM") as psA, \
             tc.tile_pool(name="psB", bufs=2, space="PSUM") as psB:

            # ---- constants
            gws_t = cst.tile([128, 40 * 128], BF)
            nc.sync.dma_start(out=gws_t[:], in_=gws_d[:])
            iot_t = cst.tile([128, 2], F32)
            nc.sync.dma_start(out=iot_t[:], in_=iot_d[:])
            scw_t = cst.tile([128, 4], BF)
            nc.sync.dma_start(out=scw_t[:], in_=scw_d[:])
            bias_t = cst.tile([128, 4], F32)
            nc.sync.dma_start(out=bias_t[:], in_=bias_d[:])
            zpad_t = cst.tile([1, 16], F32)
            nc.vector.memset(zpad_t[:], 0.0)
            nc.sync.dma_start(out=s1row_d[0:1, L:L + 16], in_=zpad_t[:])

            # ---- persistent y planes: [128, 2, HP] (even | odd)
            yd = [ydp.tile([128, 2 * HP], BF, name=f"yd{dc}", tag=f"yd{dc}")
                  for dc in range(NDC)]
            ydv = [yd[dc][:].rearrange("p (h c) -> p h c", h=2)
                   for dc in range(NDC)]
            for dc in range(NDC):
                nc.vector.memset(ydv[dc][:, :, T:HP], 0.0)

            def conv_tile(i):
                c0 = i * LTS - 2
                c1 = i * LTS + 514
                lo = max(c0, 0)
                hi = min(c1, L)
                d0 = lo - c0
                d1 = 516 - (c1 - hi)
                idst = ohp.tile([128, 516], BF, tag="idst", name=f"idst{i}")
                nc.sync.dma_start(out=idst[:, d0:d1],
                                  in_=ids_d[0:1, lo:hi].partition_broadcast(128))
                ohs = []
                for vc in range(NVC):
                    oh = ohp.tile([128, 516], BF, tag=f"oh{vc}", name=f"oh{vc}_{i}")
                    if d0 > 0:
                        nc.vector.memset(oh[:, 0:d0], 0.0)
                    if d1 < 516:
                        nc.vector.memset(oh[:, d1:516], 0.0)
                    nc.vector.tensor_scalar(out=oh[:, d0:d1], in0=idst[:, d0:d1],
                                            scalar1=iot_t[:, vc:vc + 1], scalar2=None,
                                            op0=OP.is_equal)
                    ohs.append(oh)
                for dc in range(NDC):
                    ps = psA.tile([128, LTS], F32, tag="convps", name=f"ps_{i}_{dc}")
                    for j in range(10):
                        k, vc = divmod(j, 2)
                        nc.tensor.matmul(
                            out=ps[:],
                            lhsT=gws_t[:, ((k * 2 + vc) * 4 + dc) * 128:
                                       ((k * 2 + vc) * 4 + dc) * 128 + 128],
                            rhs=ohs[vc][:, k:k + LTS],
                            start=(j == 0), stop=(j == 9))
                    # PSUM -> SBUF deinterleaved into even/odd planes
                    nc.scalar.activation(
                        out=ydv[dc][:, :, i * 256:(i + 1) * 256],
                        in_=ps[:].rearrange("p (c h) -> p h c", h=2),
                        func=mybir.ActivationFunctionType.Identity,
                        bias=bias_t[:, dc:dc + 1])
                # s1 via strided interleave view of the planes
                ps1 = psB.tile([1, LTS], F32, tag="s1ps", name=f"ps1_{i}")
                for dc in range(NDC):
                    ydt = yd[dc][:].rearrange("p (h c) -> p c h", h=2)
                    nc.tensor.matmul(out=ps1[:], lhsT=scw_t[:, dc:dc + 1],
                                     rhs=ydt[:, i * 256:(i + 1) * 256, :],
                                     start=(dc == 0), stop=(dc == NDC - 1))
                s1t = s1p.tile([1, LTS], F32, tag="s1t", name=f"s1t_{i}")
                nc.scalar.copy(out=s1t[:], in_=ps1[:])
                nc.sync.dma_start(out=s1row_d[0:1, i * LTS:(i + 1) * LTS],
                                  in_=s1t[:])

            def scores_seg(s):
                t0 = s * SEGT
                l0 = 2 * t0
                j0 = l0 // 3
                r0 = l0 - 3 * j0
                # --- candidate scores in l-major [128, 16]
                S1 = smp.tile([128, 16], F32, tag="S1", name=f"S1_{s}")
                nc.sync.dma_start(out=S1[:], in_=s1row_d[0:1, l0:l0 + SEGL])
                Tt = smp.tile([114, 18], F32, tag="Tt", name=f"Tt_{s}")
                nc.sync.dma_start(out=Tt[:],
                                  in_=s1row_d[0:1, 3 * j0:3 * j0 + 2052])
                T3 = smp.tile([114, 6], F32, tag="T3", name=f"T3_{s}")
                Tv = Tt[:].rearrange("p (c three) -> p c three", three=3)
                nc.vector.tensor_tensor(out=T3[:], in0=Tv[:, :, 0], in1=Tv[:, :, 1],
                                        op=OP.add)
                nc.vector.tensor_tensor(out=T3[:], in0=T3[:], in1=Tv[:, :, 2],
                                        op=OP.add)
                U = smp.tile([114, 18], F32, tag="U", name=f"U_{s}")
                nc.vector.tensor_copy(
                    out=U[:].rearrange("p (c three) -> p c three", three=3),
                    in_=T3[:].unsqueeze(2).to_broadcast([114, 6, 3]))
                nc.sync.dma_start(out=us3_d[0:1, s * 2052:(s + 1) * 2052], in_=U[:])
                S3c = smp.tile([128, 16], F32, tag="S3c", name=f"S3c_{s}")
                nc.sync.dma_start(out=S3c[:],
                                  in_=us3_d[0:1, s * 2052 + r0:s * 2052 + r0 + SEGL])
                s2 = smp.tile([128, 8], F32, tag="s2", name=f"s2_{s}")
                S1pair = S1[:].rearrange("p (n two) -> p n two", two=2)
                nc.vector.tensor_tensor(out=s2[:], in0=S1pair[:, :, 0],
                                        in1=S1pair[:, :, 1], op=OP.add)
                s4 = smp.tile([128, 4], F32, tag="s4", name=f"s4_{s}")
                s2pair = s2[:].rearrange("p (n two) -> p n two", two=2)
                nc.vector.tensor_tensor(out=s4[:], in0=s2pair[:, :, 0],
                                        in1=s2pair[:, :, 1], op=OP.add)
                s2h = smp.tile([128, 8], F32, tag="s2h", name=f"s2h_{s}")
                nc.vector.tensor_scalar(out=s2h[:], in0=s2[:], scalar1=0.5,
                                        scalar2=None, op0=OP.mult)
                s4q = smp.tile([128, 4], F32, tag="s4q", name=f"s4q_{s}")
                nc.vector.tensor_scalar(out=s4q[:], in0=s4[:], scalar1=0.25,
                                        scalar2=None, op0=OP.mult)
                S3s = smp.tile([128, 16], F32, tag="S3s", name=f"S3s_{s}")
                nc.vector.tensor_scalar(out=S3s[:], in0=S3c[:], scalar1=1.0 / 3.0,
                                        scalar2=None, op0=OP.mult)
                # --- softmax
                mM = smp.tile([128, 16], F32, tag="mM", name=f"mM_{s}")
                nc.vector.tensor_tensor(out=mM[:], in0=S1[:], in1=S3s[:], op=OP.max)
                nc.vector.tensor_tensor(
                    out=mM[:].rearrange("p (n two) -> p n two", two=2),
                    in0=mM[:].rearrange("p (n two) -> p n two", two=2),
                    in1=s2h[:].unsqueeze(2).to_broadcast([128, 8, 2]), op=OP.max)
                nc.vector.tensor_tensor(
                    out=mM[:].rearrange("p (n four) -> p n four", four=4),
                    in0=mM[:].rearrange("p (n four) -> p n four", four=4),
                    in1=s4q[:].unsqueeze(2).to_broadcast([128, 4, 4]), op=OP.max)
                S = smp.tile([128, 64], F32, tag="S", name=f"S_{s}")
                nc.vector.tensor_tensor(out=S[:, 0:16], in0=S1[:], in1=mM[:],
                                        op=OP.subtract)
                nc.vector.tensor_tensor(
                    out=S[:, 16:32].rearrange("p (n two) -> p n two", two=2),
                    in0=s2h[:].unsqueeze(2).to_broadcast([128, 8, 2]),
                    in1=mM[:].rearrange("p (n two) -> p n two", two=2),
                    op=OP.subtract)
                nc.vector.tensor_tensor(out=S[:, 32:48], in0=S3s[:], in1=mM[:],
                                        op=OP.subtract)
                nc.vector.tensor_tensor(
                    out=S[:, 48:64].rearrange("p (n four) -> p n four", four=4),
                    in0=s4q[:].unsqueeze(2).to_broadcast([128, 4, 4]),
                    in1=mM[:].rearrange("p (n four) -> p n four", four=4),
                    op=OP.subtract)
                nc.scalar.activation(out=S[:], in_=S[:],
                                     func=mybir.ActivationFunctionType.Exp)
                Z = smp.tile([128, 16], F32, tag="Z", name=f"Z_{s}")
                S4v = S[:].rearrange("p (four n) -> p four n", four=4)
                nc.vector.tensor_tensor(out=Z[:], in0=S4v[:, 0], in1=S4v[:, 1],
                                        op=OP.add)
                nc.vector.tensor_tensor(out=Z[:], in0=Z[:], in1=S4v[:, 2], op=OP.add)
                nc.vector.tensor_tensor(out=Z[:], in0=Z[:], in1=S4v[:, 3], op=OP.add)
                R = smp.tile([128, 16], F32, tag="R", name=f"R_{s}")
                nc.vector.reciprocal(out=R[:], in_=Z[:])
                W = smp.tile([128, 64], F32, tag="W", name=f"W_{s}")
                nc.vector.tensor_tensor(
                    out=W[:].rearrange("p (four n) -> p four n", four=4), in0=S4v,
                    in1=R[:].unsqueeze(1).to_broadcast([128, 4, 16]), op=OP.mult)
                # --- weight rows
                W1 = W[:, 0:16].rearrange("p (n two) -> p n two", two=2)
                W2 = W[:, 16:32].rearrange("p (n two) -> p n two", two=2)
                W4 = W[:, 48:64].rearrange("p (n two) -> p n two", two=2)
                c2 = smp.tile([128, 8], F32, tag="c2", name=f"c2_{s}")
                nc.vector.tensor_tensor(out=c2[:], in0=W2[:, :, 0], in1=W2[:, :, 1],
                                        op=OP.add)
                c4 = smp.tile([128, 8], F32, tag="c4", name=f"c4_{s}")
                nc.vector.tensor_tensor(out=c4[:], in0=W4[:, :, 0], in1=W4[:, :, 1],
                                        op=OP.add)
                c4s = smp.tile([128, 8], F32, tag="c4s", name=f"c4s_{s}")
                nc.vector.tensor_scalar(out=c4s[:], in0=c4[:], scalar1=0.125,
                                        scalar2=None, op0=OP.mult)
                c24 = smp.tile([128, 8], F32, tag="c24", name=f"c24_{s}")
                nc.vector.scalar_tensor_tensor(out=c24[:], in0=c2[:], scalar=0.25,
                                               in1=c4s[:], op0=OP.mult, op1=OP.add)
                we_t = smp.tile([128, 8], BF, tag="we_t", name=f"we_t_{s}")
                nc.vector.scalar_tensor_tensor(out=we_t[:], in0=W1[:, :, 0],
                                               scalar=0.5, in1=c24[:],
                                               op0=OP.mult, op1=OP.add)
                wo_t = smp.tile([128, 8], BF, tag="wo_t", name=f"wo_t_{s}")
                nc.vector.scalar_tensor_tensor(out=wo_t[:], in0=W1[:, :, 1],
                                               scalar=0.5, in1=c24[:],
                                               op0=OP.mult, op1=OP.add)
                cw4_t = smp.tile([128, 8], BF, tag="cw4_t", name=f"cw4_t_{s}")
                nc.vector.tensor_copy(out=cw4_t[:], in_=c4s[:])
                w3w = smp.tile([128, 16], BF, tag="w3w", name=f"w3w_{s}")
                nc.vector.tensor_scalar(out=w3w[:], in0=W[:, 32:48],
                                        scalar1=1.0 / 6.0, scalar2=None, op0=OP.mult)
                nc.sync.dma_start(out=we_d[0:1, t0:t0 + SEGT], in_=we_t[:])
                nc.sync.dma_start(out=wo_d[0:1, t0:t0 + SEGT], in_=wo_t[:])
                nc.sync.dma_start(out=cw4_d[0:1, t0:t0 + SEGT], in_=cw4_t[:])
                nc.sync.dma_start(out=w3row_d[0:1, l0:l0 + SEGL], in_=w3w[:])
                # --- b3 class rows from w3
                t0_, mA0, nA, mB0, nB, mD0, nD, mJ0, mHI = _seg_windows(s)
                wlo = max(0, l0 - 4)
                w3seg = rwp.tile([1, 2064], BF, tag="w3seg", name=f"w3seg_{s}")
                nc.sync.dma_start(out=w3seg[0:1, 0:l0 + SEGL - wlo],
                                  in_=w3row_d[0:1, wlo:l0 + SEGL])
                cwa_t = rwp.tile([1, 348], BF, tag="cwa_t", name=f"cwa_t_{s}")
                a0 = 6 * mA0 - wlo
                nc.vector.tensor_tensor(out=cwa_t[0:1, 0:nA],
                                        in0=_sl1(w3seg, a0, nA, 6),
                                        in1=_sl1(w3seg, a0 + 1, nA, 6),
                                        op=OP.add)
                cwb_t = rwp.tile([1, 348], BF, tag="cwb_t", name=f"cwb_t_{s}")
                b0 = 6 * mB0 + 2 - wlo
                nc.vector.tensor_copy(out=cwb_t[0:1, 0:nB],
                                      in_=_sl1(w3seg, b0, nB, 6))
                cwc_t = rwp.tile([1, 348], BF, tag="cwc_t", name=f"cwc_t_{s}")
                nc.vector.tensor_copy(out=cwc_t[0:1, 0:nB],
                                      in_=_sl1(w3seg, b0 + 1, nB, 6))
                cwd_t = rwp.tile([1, 348], BF, tag="cwd_t", name=f"cwd_t_{s}")
                d0_ = 6 * mD0 + 4 - wlo
                nc.vector.tensor_tensor(out=cwd_t[0:1, 0:nD],
                                        in0=_sl1(w3seg, d0_, nD, 6),
                                        in1=_sl1(w3seg, d0_ + 1, nD, 6),
                                        op=OP.add)
                for cl, tl, n in ((0, cwa_t, nA), (1, cwb_t, nB), (2, cwc_t, nB),
                                  (3, cwd_t, nD)):
                    nc.sync.dma_start(out=cwrow_d[cl][0:1, s * 348:s * 348 + n],
                                      in_=tl[0:1, 0:n])

            def combine_seg(s):
                t0, mA0, nA, mB0, nB, mD0, nD, mJ0, mHI = _seg_windows(s)
                nJ = mHI - mJ0
                we_s = wsg.tile([128, SEGT], BF, tag="we_s", name=f"we_s_{s}")
                nc.sync.dma_start(
                    out=we_s[:], in_=we_d[0:1, t0:t0 + SEGT].partition_broadcast(128))
                wo_s = wsg.tile([128, SEGT], BF, tag="wo_s", name=f"wo_s_{s}")
                nc.sync.dma_start(
                    out=wo_s[:], in_=wo_d[0:1, t0:t0 + SEGT].partition_broadcast(128))
                cw4_s = wsg.tile([128, SEGT], BF, tag="cw4_s", name=f"cw4_s_{s}")
                nc.sync.dma_start(
                    out=cw4_s[:],
                    in_=cw4_d[0:1, t0:t0 + SEGT].partition_broadcast(128))
                cw_s = []
                for cl, n in ((0, nA), (1, nB), (2, nB), (3, nD)):
                    t_ = wsg.tile([128, 348], BF, tag=f"cw{cl}_s",
                                  name=f"cw{cl}_s_{s}")
                    nc.sync.dma_start(
                        out=t_[:, 0:n],
                        in_=cwrow_d[cl][0:1, s * 348:s * 348 + n]
                        .partition_broadcast(128))
                    cw_s.append(t_)

                ov = out_d[:].rearrange("(tb p) (dc c) -> p tb dc c", p=128, c=128)
                for dc in range(NDC):
                    ye = ydv[dc][:, 0, :]
                    yo = ydv[dc][:, 1, :]
                    # p2 over [t0-2, t0+1026)
                    p2 = cmb.tile([128, 1028], BF, tag="p2", name=f"p2_{s}_{dc}")
                    if s == 0:
                        nc.vector.memset(p2[:, 0:2], 0.0)
                        nc.vector.tensor_tensor(out=p2[:, 2:1028],
                                                in0=ye[:, 0:1026],
                                                in1=yo[:, 0:1026], op=OP.add)
                    else:
                        nc.vector.tensor_tensor(out=p2[:, 0:1028],
                                                in0=ye[:, t0 - 2:t0 + 1026],
                                                in1=yo[:, t0 - 2:t0 + 1026],
                                                op=OP.add)
                    m_e = ctp.tile([128, SEGT], BF, tag="ct", name=f"me_{s}_{dc}")
                    nc.vector.tensor_tensor(out=m_e[:], in0=ye[:, t0:t0 + SEGT],
                                            in1=we_s[:], op=OP.mult)
                    m_o = ctp.tile([128, SEGT], BF, tag="ct", name=f"mo_{s}_{dc}")
                    nc.vector.tensor_tensor(out=m_o[:], in0=yo[:, t0:t0 + SEGT],
                                            in1=wo_s[:], op=OP.mult)
                    acc = cmb.tile([128, SEGT], BF, tag="acc", name=f"acc_{s}_{dc}")
                    nc.vector.tensor_tensor(out=acc[:], in0=m_e[:], in1=m_o[:],
                                            op=OP.add)
                    # b4 residue: cw4[t] * p2[swap(t)]
                    s4t = ctp.tile([128, SEGT], BF, tag="ct", name=f"s4t_{s}_{dc}")
                    p2sw = p2[:, 2:1026].rearrange("p (v two) -> p v two",
                                                   two=2)[:, :, ::-1]
                    nc.vector.tensor_tensor(
                        out=s4t[:].rearrange("p (v two) -> p v two", two=2),
                        in0=p2sw,
                        in1=cw4_s[:].rearrange("p (v two) -> p v two", two=2),
                        op=OP.mult)
                    nc.vector.tensor_tensor(out=acc[:], in0=acc[:], in1=s4t[:],
                                            op=OP.add)
                    # b3 pooled planes on the m-grid [mJ0, mHI)
                    p3e = cmb.tile([128, 348], BF, tag="p3e", name=f"p3e_{s}_{dc}")
                    e0 = 3 * mJ0 - (t0 - 2)
                    nc.vector.tensor_tensor(
                        out=p3e[:, 0:nJ],
                        in0=_sl(p2, e0, nJ, 3),
                        in1=_sl(ye, 3 * mJ0 + 1, nJ, 3), op=OP.add)
                    p3o = cmb.tile([128, 348], BF, tag="p3o", name=f"p3o_{s}_{dc}")
                    nc.vector.tensor_tensor(
                        out=p3o[:, 0:nJ],
                        in0=_sl(yo, 3 * mJ0 + 1, nJ, 3),
                        in1=_sl(p2, e0 + 2, nJ, 3), op=OP.add)
                    tb = []
                    for bi, (p3t, m0, n, cwt) in enumerate((
                            (p3e, mA0, nA, cw_s[0]), (p3e, mB0, nB, cw_s[1]),
                            (p3o, mB0, nB, cw_s[2]), (p3o, mD0, nD, cw_s[3]))):
                        t_ = otp.tile([128, 348], BF, tag="tb",
                                      name=f"tb{bi}_{s}_{dc}")
                        o = m0 - mJ0
                        nc.vector.tensor_tensor(out=t_[:, 0:n],
                                                in0=p3t[:, o:o + n],
                                                in1=cwt[:, 0:n], op=OP.mult)
                        tb.append(t_)
                    X = otp.tile([128, 348], BF, tag="tb", name=f"X_{s}_{dc}")
                    nc.vector.tensor_tensor(out=X[:, 0:nB], in0=tb[1][:, 0:nB],
                                            in1=tb[2][:, 0:nB], op=OP.add)
                    for (t_, m0, n, coff) in ((tb[0], mA0, nA, 0), (X, mB0, nB, 1),
                                              (tb[3], mD0, nD, 2)):
                        st = 3 * m0 + coff - t0
                        av = _sl(acc, st, n, 3)
                        nc.vector.tensor_tensor(out=av, in0=av, in1=t_[:, 0:n],
                                                op=OP.add)
                    # transpose via DMA xbar and store
                    otr = otp.tile([128, 8 * 128], BF, tag="otr",
                                   name=f"otr_{s}_{dc}")
                    nc.sync.dma_start_transpose(
                        out=otr[:].rearrange("p (tb c) -> p tb c", c=128),
                        in_=acc[:])
                    nc.sync.dma_start(
                        out=ov[:, s * 8:(s + 1) * 8, dc, :],
                        in_=otr[:].rearrange("p (tb c) -> p tb c", c=128))

            # ---- emission: pipelined
            for i in range(5):
                conv_tile(i)
            for s in range(NSEG):
                scores_seg(s)
                for i in range(4 * s + 5, min(4 * s + 9, NLT)):
                    conv_tile(i)
                combine_seg(s)
    nc.compile()
    return nc


def _get_nc():
    global _NC
    if _NC is None:
        _NC = _build()
    return _NC


def kernel(input_ids, emb, conv_w, conv_b, score_w):
    global LAST_RESULT
    nc = _get_nc()
    input_ids = np.asarray(input_ids)
    emb = np.asarray(emb, dtype=np.float32)
    conv_w = np.asarray(conv_w, dtype=np.float32)
    conv_b = np.asarray(conv_b, dtype=np.float32)
    score_w = np.asarray(score_w, dtype=np.float32)
    B = input_ids.shape[0]

    G = np.einsum("oik,vi->kvo", conv_w.astype(np.float64),
                  emb.astype(np.float64)).astype(np.float32)  # [K, V, D]
    gws = np.zeros((128, 40, 128), np.float32)
    for k in range(K):
        for vc in range(NVC):
            for dc in range(NDC):
                gws[:, (k * 2 + vc) * 4 + dc, :] = \
                    G[k, vc * 128:(vc + 1) * 128, dc * 128:(dc + 1) * 128]
    gws = gws.reshape(128, 40 * 128).astype(bf16)
    iot = np.stack([np.arange(128), np.arange(128) + 128], axis=1).astype(np.float32)
    scw = score_w.reshape(4, 128).T.astype(bf16)
    biasm = conv_b.reshape(4, 128).T.astype(np.float32)
    idsb = input_ids.astype(np.float32).astype(bf16)

    in_maps = [{"ids": np.ascontiguousarray(idsb[c:c + 1]), "gws": gws, "iot": iot,
                "scw": scw, "bias": biasm} for c in range(B)]
    res = run_bass_kernel_spmd(nc, in_maps, core_ids=list(range(B)), trace=TRACE)
    LAST_RESULT = res
    return np.stack([np.asarray(res.results[c]["out"]).astype(np.float32)
                     for c in range(B)])


# revision 3
# speedup vs baseline: 1.4160x; 1.0041x over previous
"""GBST embedding kernel for Trainium2, data-parallel over batch on 8 cores.

v2: pipelined per-segment structure.
- Conv folded with embedding gather via onehot matmul (vocab contraction),
  y stored in even/odd deinterleaved planes (unit-stride DVE combine).
- b2+b4 combine terms folded into per-t row weights we/wo (one product per
  y plane); b4 residue handled via a reversed-pair strided view of p2.
- b3 handled on the u-grid with 4 residue-class rows, products unit-stride.
- Softmax per 2048-l segment in l-major [128,16] layout; s3 pooling done
  multi-partition via a [114,18] reshape trick.
- Output written bf16 via DMA xbar transpose (no PE transposes); host
  converts to f32.
"""
import sys
sys.path.insert(0, "/opt/trn_rl_repo")
import numpy as np
import ml_dtypes

import concourse.bass as bass
import concourse.bacc as bacc
import concourse.tile as tile
from concourse import mybir
from concourse.bass_utils import run_bass_kernel_spmd

bf16 = ml_dtypes.bfloat16
F32 = mybir.dt.float32
BF = mybir.dt.bfloat16
OP = mybir.AluOpType

L, T, V, D, K = 8192, 4096, 256, 512, 5
NDC, NVC, NLT, LTS = 4, 2, 16, 512
NSEG, SEGT, SEGL = 4, 1024, 2048
HP = 4104          # per-parity plane width (4096 + 8 pad)

TRACE = False
LAST_RESULT = None
_NC = None


def _seg_windows(s):
    """Per-seg b3 class windows on the m-grid (t=3m/3m+1/3m+2)."""
    t0, t1 = s * SEGT, (s + 1) * SEGT
    mA0 = -(-t0 // 3)
    nA = len(range(3 * mA0, t1, 3))
    mB0 = -(-(t0 - 1) // 3)
    nB = len(range(3 * mB0 + 1, t1, 3))
    mD0 = -(-(t0 - 2) // 3)
    nD = len(range(3 * mD0 + 2, t1, 3))
    mJ0 = min(mA0, mB0, mD0)
    mHI = max(mA0 + nA, mB0 + nB, mD0 + nD)
    return t0, mA0, nA, mB0, nB, mD0, nD, mJ0, mHI




def _sl(ap, start, n, step):
    """Tight strided slice of n elements from an AP's free dim."""
    return ap[:, start:start + step * (n - 1) + 1:step]


def _sl1(ap, start, n, step):
    return ap[0:1, start:start + step * (n - 1) + 1:step]


def _build():
    nc = bacc.Bacc("TRN2", target_bir_lowering=False)
    ids_d = nc.dram_tensor("ids", [1, L], BF, kind="ExternalInput")
    gws_d = nc.dram_tensor("gws", [128, 40 * 128], BF, kind="ExternalInput")
    iot_d = nc.dram_tensor("iot", [128, 2], F32, kind="ExternalInput")
    scw_d = nc.dram_tensor("scw", [128, 4], BF, kind="ExternalInput")
    bias_d = nc.dram_tensor("bias", [128, 4], F32, kind="ExternalInput")
    out_d = nc.dram_tensor("out", [T, D], BF, kind="ExternalOutput")
    # DRAM staging
    s1row_d = nc.dram_tensor("s1row_d", [1, L + 16], F32)
    us3_d = nc.dram_tensor("us3_d", [1, NSEG * 2052], F32)
    w3row_d = nc.dram_tensor("w3row_d", [1, L + 16], BF)
    we_d = nc.dram_tensor("we_d", [1, T], BF)
    wo_d = nc.dram_tensor("wo_d", [1, T], BF)
    cw4_d = nc.dram_tensor("cw4_d", [1, T], BF)
    cwrow_d = [nc.dram_tensor(f"cw{cl}_d", [1, NSEG * 348], BF)
               for cl in "abcd"]

    with tile.TileContext(nc) as tc:
        with tc.tile_pool(name="const", bufs=1) as cst, \
             tc.tile_pool(name="ydp", bufs=1) as ydp, \
             tc.tile_pool(name="ohp", bufs=2) as ohp, \
             tc.tile_pool(name="s1p", bufs=2) as s1p, \
             tc.tile_pool(name="smp", bufs=2) as smp, \
             tc.tile_pool(name="rwp", bufs=2) as rwp, \
             tc.tile_pool(name="wsg", bufs=2) as wsg, \
             tc.tile_pool(name="cmb", bufs=2) as cmb, \
             tc.tile_pool(name="accp", bufs=8) as accp, \
             tc.tile_pool(name="ctp", bufs=3) as ctp, \
             tc.tile_pool(name="otp", bufs=3) as otp, \
             tc.tile_pool(name="psA", bufs=3, space="PSUM") as psA, \
             tc.tile_pool(name="psB", bufs=2, space="PSUM") as psB:

            # ---- constants
            gws_t = cst.tile([128, 40 * 128], BF)
            nc.sync.dma_start(out=gws_t[:], in_=gws_d[:])
            iot_t = cst.tile([128, 2], F32)
            nc.sync.dma_start(out=iot_t[:], in_=iot_d[:])
            scw_t = cst.tile([128, 4], BF)
            nc.sync.dma_start(out=scw_t[:], in_=scw_d[:])
            bias_t = cst.tile([128, 4], F32)
            nc.sync.dma_start(out=bias_t[:], in_=bias_d[:])
            zpad_t = cst.tile([1, 16], F32)
            nc.vector.memset(zpad_t[:], 0.0)
            nc.sync.dma_start(out=s1row_d[0:1, L:L + 16], in_=zpad_t[:])

            # ---- persistent y planes: [128, 2, HP] (even | odd)
            yd = [ydp.tile([128, 2 * HP], BF, name=f"yd{dc}", tag=f"yd{dc}")
                  for dc in range(NDC)]
            ydv = [yd[dc][:].rearrange("p (h c) -> p h c", h=2)
                   for dc in range(NDC)]
            for dc in range(NDC):
                nc.vector.memset(ydv[dc][:, :, T:HP], 0.0)

            def conv_tile(i):
                c0 = i * LTS - 2
                c1 = i * LTS + 514
                lo = max(c0, 0)
                hi = min(c1, L)
                d0 = lo - c0
                d1 = 516 - (c1 - hi)
                idst = ohp.tile([128, 516], BF, tag="idst", name=f"idst{i}")
                nc.sync.dma_start(out=idst[:, d0:d1],
                                  in_=ids_d[0:1, lo:hi].partition_broadcast(128))
                ohs = []
                for vc in range(NVC):
                    oh = ohp.tile([128, 516], BF, tag=f"oh{vc}", name=f"oh{vc}_{i}")
                    if d0 > 0:
                        nc.vector.memset(oh[:, 0:d0], 0.0)
                    if d1 < 516:
                        nc.vector.memset(oh[:, d1:516], 0.0)
                    nc.vector.tensor_scalar(out=oh[:, d0:d1], in0=idst[:, d0:d1],
                                            scalar1=iot_t[:, vc:vc + 1], scalar2=None,
                                            op0=OP.is_equal)
                    ohs.append(oh)
                for dc in range(NDC):
                    ps = psA.tile([128, LTS], F32, tag="convps", name=f"ps_{i}_{dc}")
                    for j in range(10):
                        k, vc = divmod(j, 2)
                        nc.tensor.matmul(
                            out=ps[:],
                            lhsT=gws_t[:, ((k * 2 + vc) * 4 + dc) * 128:
                                       ((k * 2 + vc) * 4 + dc) * 128 + 128],
                            rhs=ohs[vc][:, k:k + LTS],
                            start=(j == 0), stop=(j == 9))
                    # PSUM -> SBUF deinterleaved into even/odd planes
                    nc.scalar.activation(
                        out=ydv[dc][:, :, i * 256:(i + 1) * 256],
                        in_=ps[:].rearrange("p (c h) -> p h c", h=2),
                        func=mybir.ActivationFunctionType.Identity,
                        bias=bias_t[:, dc:dc + 1])
                # s1 with parity-major psum ([s1e | s1o]); interleave on copy
                ps1 = psB.tile([1, LTS], F32, tag="s1ps", name=f"ps1_{i}")
                for dc in range(NDC):
                    nc.tensor.matmul(out=ps1[:], lhsT=scw_t[:, dc:dc + 1],
                                     rhs=ydv[dc][:, :, i * 256:(i + 1) * 256],
                                     start=(dc == 0), stop=(dc == NDC - 1))
                s1t = s1p.tile([1, LTS], F32, tag="s1t", name=f"s1t_{i}")
                nc.scalar.copy(out=s1t[:].rearrange("p (c h) -> p h c", h=2),
                               in_=ps1[:].rearrange("p (h c) -> p h c", h=2))
                nc.sync.dma_start(out=s1row_d[0:1, i * LTS:(i + 1) * LTS],
                                  in_=s1t[:])

            def scores_seg(s):
                t0 = s * SEGT
                l0 = 2 * t0
                j0 = l0 // 3
                r0 = l0 - 3 * j0
                # --- candidate scores in l-major [128, 16]
                S1 = smp.tile([128, 16], F32, tag="S1", name=f"S1_{s}")
                nc.sync.dma_start(out=S1[:], in_=s1row_d[0:1, l0:l0 + SEGL])
                Tt = smp.tile([114, 18], F32, tag="Tt", name=f"Tt_{s}")
                nc.sync.dma_start(out=Tt[:],
                                  in_=s1row_d[0:1, 3 * j0:3 * j0 + 2052])
                T3 = smp.tile([114, 6], F32, tag="T3", name=f"T3_{s}")
                Tv = Tt[:].rearrange("p (c three) -> p c three", three=3)
                nc.vector.tensor_tensor(out=T3[:], in0=Tv[:, :, 0], in1=Tv[:, :, 1],
                                        op=OP.add)
                nc.vector.tensor_tensor(out=T3[:], in0=T3[:], in1=Tv[:, :, 2],
                                        op=OP.add)
                U = smp.tile([114, 18], F32, tag="U", name=f"U_{s}")
                nc.vector.tensor_copy(
                    out=U[:].rearrange("p (c three) -> p c three", three=3),
                    in_=T3[:].unsqueeze(2).to_broadcast([114, 6, 3]))
                nc.sync.dma_start(out=us3_d[0:1, s * 2052:(s + 1) * 2052], in_=U[:])
                S3c = smp.tile([128, 16], F32, tag="S3c", name=f"S3c_{s}")
                nc.sync.dma_start(out=S3c[:],
                                  in_=us3_d[0:1, s * 2052 + r0:s * 2052 + r0 + SEGL])
                s2 = smp.tile([128, 8], F32, tag="s2", name=f"s2_{s}")
                S1pair = S1[:].rearrange("p (n two) -> p n two", two=2)
                nc.vector.tensor_tensor(out=s2[:], in0=S1pair[:, :, 0],
                                        in1=S1pair[:, :, 1], op=OP.add)
                s4 = smp.tile([128, 4], F32, tag="s4", name=f"s4_{s}")
                s2pair = s2[:].rearrange("p (n two) -> p n two", two=2)
                nc.vector.tensor_tensor(out=s4[:], in0=s2pair[:, :, 0],
                                        in1=s2pair[:, :, 1], op=OP.add)
                s2h = smp.tile([128, 8], F32, tag="s2h", name=f"s2h_{s}")
                nc.vector.tensor_scalar(out=s2h[:], in0=s2[:], scalar1=0.5,
                                        scalar2=None, op0=OP.mult)
                s4q = smp.tile([128, 4], F32, tag="s4q", name=f"s4q_{s}")
                nc.vector.tensor_scalar(out=s4q[:], in0=s4[:], scalar1=0.25,
                                        scalar2=None, op0=OP.mult)
                S3s = smp.tile([128, 16], F32, tag="S3s", name=f"S3s_{s}")
                nc.vector.tensor_scalar(out=S3s[:], in0=S3c[:], scalar1=1.0 / 3.0,
                                        scalar2=None, op0=OP.mult)
                # --- softmax
                mM = smp.tile([128, 16], F32, tag="mM", name=f"mM_{s}")
                nc.vector.tensor_tensor(out=mM[:], in0=S1[:], in1=S3s[:], op=OP.max)
                nc.vector.tensor_tensor(
                    out=mM[:].rearrange("p (n two) -> p n two", two=2),
                    in0=mM[:].rearrange("p (n two) -> p n two", two=2),
                    in1=s2h[:].unsqueeze(2).to_broadcast([128, 8, 2]), op=OP.max)
                nc.vector.tensor_tensor(
                    out=mM[:].rearrange("p (n four) -> p n four", four=4),
                    in0=mM[:].rearrange("p (n four) -> p n four", four=4),
                    in1=s4q[:].unsqueeze(2).to_broadcast([128, 4, 4]), op=OP.max)
                S = smp.tile([128, 64], F32, tag="S", name=f"S_{s}")
                nc.vector.tensor_tensor(out=S[:, 0:16], in0=S1[:], in1=mM[:],
                                        op=OP.subtract)
                nc.vector.tensor_tensor(
                    out=S[:, 16:32].rearrange("p (n two) -> p n two", two=2),
                    in0=s2h[:].unsqueeze(2).to_broadcast([128, 8, 2]),
                    in1=mM[:].rearrange("p (n two) -> p n two", two=2),
                    op=OP.subtract)
                nc.vector.tensor_tensor(out=S[:, 32:48], in0=S3s[:], in1=mM[:],
                                        op=OP.subtract)
                nc.vector.tensor_tensor(
                    out=S[:, 48:64].rearrange("p (n four) -> p n four", four=4),
                    in0=s4q[:].unsqueeze(2).to_broadcast([128, 4, 4]),
                    in1=mM[:].rearrange("p (n four) -> p n four", four=4),
                    op=OP.subtract)
                nc.scalar.activation(out=S[:], in_=S[:],
                                     func=mybir.ActivationFunctionType.Exp)
                Z = smp.tile([128, 16], F32, tag="Z", name=f"Z_{s}")
                S4v = S[:].rearrange("p (four n) -> p four n", four=4)
                nc.vector.tensor_tensor(out=Z[:], in0=S4v[:, 0], in1=S4v[:, 1],
                                        op=OP.add)
                nc.vector.tensor_tensor(out=Z[:], in0=Z[:], in1=S4v[:, 2], op=OP.add)
                nc.vector.tensor_tensor(out=Z[:], in0=Z[:], in1=S4v[:, 3], op=OP.add)
                R = smp.tile([128, 16], F32, tag="R", name=f"R_{s}")
                nc.vector.reciprocal(out=R[:], in_=Z[:])
                W = smp.tile([128, 64], F32, tag="W", name=f"W_{s}")
                nc.vector.tensor_tensor(
                    out=W[:].rearrange("p (four n) -> p four n", four=4), in0=S4v,
                    in1=R[:].unsqueeze(1).to_broadcast([128, 4, 16]), op=OP.mult)
                # --- weight rows
                W1 = W[:, 0:16].rearrange("p (n two) -> p n two", two=2)
                W2 = W[:, 16:32].rearrange("p (n two) -> p n two", two=2)
                W4 = W[:, 48:64].rearrange("p (n two) -> p n two", two=2)
                c2 = smp.tile([128, 8], F32, tag="c2", name=f"c2_{s}")
                nc.vector.tensor_tensor(out=c2[:], in0=W2[:, :, 0], in1=W2[:, :, 1],
                                        op=OP.add)
                c4 = smp.tile([128, 8], F32, tag="c4", name=f"c4_{s}")
                nc.vector.tensor_tensor(out=c4[:], in0=W4[:, :, 0], in1=W4[:, :, 1],
                                        op=OP.add)
                c4s = smp.tile([128, 8], F32, tag="c4s", name=f"c4s_{s}")
                nc.vector.tensor_scalar(out=c4s[:], in0=c4[:], scalar1=0.125,
                                        scalar2=None, op0=OP.mult)
                c24 = smp.tile([128, 8], F32, tag="c24", name=f"c24_{s}")
                nc.vector.scalar_tensor_tensor(out=c24[:], in0=c2[:], scalar=0.25,
                                               in1=c4s[:], op0=OP.mult, op1=OP.add)
                we_t = smp.tile([128, 8], BF, tag="we_t", name=f"we_t_{s}")
                nc.vector.scalar_tensor_tensor(out=we_t[:], in0=W1[:, :, 0],
                                               scalar=0.5, in1=c24[:],
                                               op0=OP.mult, op1=OP.add)
                wo_t = smp.tile([128, 8], BF, tag="wo_t", name=f"wo_t_{s}")
                nc.vector.scalar_tensor_tensor(out=wo_t[:], in0=W1[:, :, 1],
                                               scalar=0.5, in1=c24[:],
                                               op0=OP.mult, op1=OP.add)
                cw4_t = smp.tile([128, 8], BF, tag="cw4_t", name=f"cw4_t_{s}")
                nc.vector.tensor_copy(out=cw4_t[:], in_=c4s[:])
                w3w = smp.tile([128, 16], BF, tag="w3w", name=f"w3w_{s}")
                nc.vector.tensor_scalar(out=w3w[:], in0=W[:, 32:48],
                                        scalar1=1.0 / 6.0, scalar2=None, op0=OP.mult)
                nc.sync.dma_start(out=we_d[0:1, t0:t0 + SEGT], in_=we_t[:])
                nc.sync.dma_start(out=wo_d[0:1, t0:t0 + SEGT], in_=wo_t[:])
                nc.sync.dma_start(out=cw4_d[0:1, t0:t0 + SEGT], in_=cw4_t[:])
                nc.sync.dma_start(out=w3row_d[0:1, l0:l0 + SEGL], in_=w3w[:])
                # --- b3 class rows from w3
                t0_, mA0, nA, mB0, nB, mD0, nD, mJ0, mHI = _seg_windows(s)
                wlo = max(0, l0 - 4)
                w3seg = rwp.tile([1, 2064], BF, tag="w3seg", name=f"w3seg_{s}")
                nc.sync.dma_start(out=w3seg[0:1, 0:l0 + SEGL - wlo],
                                  in_=w3row_d[0:1, wlo:l0 + SEGL])
                cwa_t = rwp.tile([1, 348], BF, tag="cwa_t", name=f"cwa_t_{s}")
                a0 = 6 * mA0 - wlo
                nc.vector.tensor_tensor(out=cwa_t[0:1, 0:nA],
                                        in0=_sl1(w3seg, a0, nA, 6),
                                        in1=_sl1(w3seg, a0 + 1, nA, 6),
                                        op=OP.add)
                cwb_t = rwp.tile([1, 348], BF, tag="cwb_t", name=f"cwb_t_{s}")
                b0 = 6 * mB0 + 2 - wlo
                nc.vector.tensor_copy(out=cwb_t[0:1, 0:nB],
                                      in_=_sl1(w3seg, b0, nB, 6))
                cwc_t = rwp.tile([1, 348], BF, tag="cwc_t", name=f"cwc_t_{s}")
                nc.vector.tensor_copy(out=cwc_t[0:1, 0:nB],
                                      in_=_sl1(w3seg, b0 + 1, nB, 6))
                cwd_t = rwp.tile([1, 348], BF, tag="cwd_t", name=f"cwd_t_{s}")
                d0_ = 6 * mD0 + 4 - wlo
                nc.vector.tensor_tensor(out=cwd_t[0:1, 0:nD],
                                        in0=_sl1(w3seg, d0_, nD, 6),
                                        in1=_sl1(w3seg, d0_ + 1, nD, 6),
                                        op=OP.add)
                for cl, tl, n in ((0, cwa_t, nA), (1, cwb_t, nB), (2, cwc_t, nB),
                                  (3, cwd_t, nD)):
                    nc.sync.dma_start(out=cwrow_d[cl][0:1, s * 348:s * 348 + n],
                                      in_=tl[0:1, 0:n])

            ov = out_d[:].rearrange("(tb p) (dc c) -> p tb dc c", p=128, c=128)
            pend = {}

            def flush_seg(s):
                for dc in range(NDC):
                    acc = pend.pop((s, dc))
                    otr = otp.tile([128, 8 * 128], BF, tag="otr",
                                   name=f"otr_{s}_{dc}")
                    nc.sync.dma_start_transpose(
                        out=otr[:].rearrange("p (tb c) -> p tb c", c=128),
                        in_=acc[:])
                    nc.sync.dma_start(
                        out=ov[:, s * 8:(s + 1) * 8, dc, :],
                        in_=otr[:].rearrange("p (tb c) -> p tb c", c=128))

            def combine_seg(s):
                t0, mA0, nA, mB0, nB, mD0, nD, mJ0, mHI = _seg_windows(s)
                nJ = mHI - mJ0
                we_s = wsg.tile([128, SEGT], BF, tag="we_s", name=f"we_s_{s}")
                nc.sync.dma_start(
                    out=we_s[:], in_=we_d[0:1, t0:t0 + SEGT].partition_broadcast(128))
                wo_s = wsg.tile([128, SEGT], BF, tag="wo_s", name=f"wo_s_{s}")
                nc.sync.dma_start(
                    out=wo_s[:], in_=wo_d[0:1, t0:t0 + SEGT].partition_broadcast(128))
                cw4_s = wsg.tile([128, SEGT], BF, tag="cw4_s", name=f"cw4_s_{s}")
                nc.sync.dma_start(
                    out=cw4_s[:],
                    in_=cw4_d[0:1, t0:t0 + SEGT].partition_broadcast(128))
                cw_s = []
                for cl, n in ((0, nA), (1, nB), (2, nB), (3, nD)):
                    t_ = wsg.tile([128, 348], BF, tag=f"cw{cl}_s",
                                  name=f"cw{cl}_s_{s}")
                    nc.sync.dma_start(
                        out=t_[:, 0:n],
                        in_=cwrow_d[cl][0:1, s * 348:s * 348 + n]
                        .partition_broadcast(128))
                    cw_s.append(t_)

                for dc in range(NDC):
                    ye = ydv[dc][:, 0, :]
                    yo = ydv[dc][:, 1, :]
                    # p2 over [t0-2, t0+1026)
                    p2 = cmb.tile([128, 1028], BF, tag="p2", name=f"p2_{s}_{dc}")
                    if s == 0:
                        nc.vector.memset(p2[:, 0:2], 0.0)
                        nc.vector.tensor_tensor(out=p2[:, 2:1028],
                                                in0=ye[:, 0:1026],
                                                in1=yo[:, 0:1026], op=OP.add)
                    else:
                        nc.vector.tensor_tensor(out=p2[:, 0:1028],
                                                in0=ye[:, t0 - 2:t0 + 1026],
                                                in1=yo[:, t0 - 2:t0 + 1026],
                                                op=OP.add)
                    m_e = ctp.tile([128, SEGT], BF, tag="ct", name=f"me_{s}_{dc}")
                    nc.vector.tensor_tensor(out=m_e[:], in0=ye[:, t0:t0 + SEGT],
                                            in1=we_s[:], op=OP.mult)
                    m_o = ctp.tile([128, SEGT], BF, tag="ct", name=f"mo_{s}_{dc}")
                    nc.vector.tensor_tensor(out=m_o[:], in0=yo[:, t0:t0 + SEGT],
                                            in1=wo_s[:], op=OP.mult)
                    acc = accp.tile([128, SEGT], BF, tag="acc",
                                    name=f"acc_{s}_{dc}")
                    nc.vector.tensor_tensor(out=acc[:], in0=m_e[:], in1=m_o[:],
                                            op=OP.add)
                    # b4 residue: cw4[t] * p2[swap(t)]
                    s4t = ctp.tile([128, SEGT], BF, tag="ct", name=f"s4t_{s}_{dc}")
                    p2sw = p2[:, 2:1026].rearrange("p (v two) -> p v two",
                                                   two=2)[:, :, ::-1]
                    nc.vector.tensor_tensor(
                        out=s4t[:].rearrange("p (v two) -> p v two", two=2),
                        in0=p2sw,
                        in1=cw4_s[:].rearrange("p (v two) -> p v two", two=2),
                        op=OP.mult)
                    nc.vector.tensor_tensor(out=acc[:], in0=acc[:], in1=s4t[:],
                                            op=OP.add)
                    # b3 pooled planes on the m-grid [mJ0, mHI)
                    p3e = cmb.tile([128, 348], BF, tag="p3e", name=f"p3e_{s}_{dc}")
                    e0 = 3 * mJ0 - (t0 - 2)
                    nc.vector.tensor_tensor(
                        out=p3e[:, 0:nJ],
                        in0=_sl(p2, e0, nJ, 3),
                        in1=_sl(ye, 3 * mJ0 + 1, nJ, 3), op=OP.add)
                    p3o = cmb.tile([128, 348], BF, tag="p3o", name=f"p3o_{s}_{dc}")
                    nc.vector.tensor_tensor(
                        out=p3o[:, 0:nJ],
                        in0=_sl(yo, 3 * mJ0 + 1, nJ, 3),
                        in1=_sl(p2, e0 + 2, nJ, 3), op=OP.add)
                    tb = []
                    for bi, (p3t, m0, n, cwt) in enumerate((
                            (p3e, mA0, nA, cw_s[0]), (p3e, mB0, nB, cw_s[1]),
                            (p3o, mB0, nB, cw_s[2]), (p3o, mD0, nD, cw_s[3]))):
                        t_ = otp.tile([128, 348], BF, tag="tb",
                                      name=f"tb{bi}_{s}_{dc}")
                        o = m0 - mJ0
                        nc.vector.tensor_tensor(out=t_[:, 0:n],
                                                in0=p3t[:, o:o + n],
                                                in1=cwt[:, 0:n], op=OP.mult)
                        tb.append(t_)
                    X = otp.tile([128, 348], BF, tag="tb", name=f"X_{s}_{dc}")
                    nc.vector.tensor_tensor(out=X[:, 0:nB], in0=tb[1][:, 0:nB],
                                            in1=tb[2][:, 0:nB], op=OP.add)
                    for (t_, m0, n, coff) in ((tb[0], mA0, nA, 0), (X, mB0, nB, 1),
                                              (tb[3], mD0, nD, 2)):
                        st = 3 * m0 + coff - t0
                        av = _sl(acc, st, n, 3)
                        nc.vector.tensor_tensor(out=av, in0=av, in1=t_[:, 0:n],
                                                op=OP.add)
                    pend[(s, dc)] = acc

            # ---- emission: pipelined, transposes deferred one segment
            for i in range(5):
                conv_tile(i)
            for s in range(NSEG):
                scores_seg(s)
                if s >= 1:
                    flush_seg(s - 1)
                for i in range(4 * s + 5, min(4 * s + 9, NLT)):
                    conv_tile(i)
                combine_seg(s)
            flush_seg(NSEG - 1)
    nc.compile()
    return nc


def _get_nc():
    global _NC
    if _NC is None:
        _NC = _build()
    return _NC


def kernel(input_ids, emb, conv_w, conv_b, score_w):
    global LAST_RESULT
    nc = _get_nc()
    input_ids = np.asarray(input_ids)
    emb = np.asarray(emb, dtype=np.float32)
    conv_w = np.asarray(conv_w, dtype=np.float32)
    conv_b = np.asarray(conv_b, dtype=np.float32)
    score_w = np.asarray(score_w, dtype=np.float32)
    B = input_ids.shape[0]

    G = np.einsum("oik,vi->kvo", conv_w.astype(np.float64),
                  emb.astype(np.float64)).astype(np.float32)  # [K, V, D]
    gws = np.zeros((128, 40, 128), np.float32)
    for k in range(K):
        for vc in range(NVC):
            for dc in range(NDC):
                gws[:, (k * 2 + vc) * 4 + dc, :] = \
                    G[k, vc * 128:(vc + 1) * 128, dc * 128:(dc + 1) * 128]
    gws = gws.reshape(128, 40 * 128).astype(bf16)
    iot = np.stack([np.arange(128), np.arange(128) + 128], axis=1).astype(np.float32)
    scw = score_w.reshape(4, 128).T.astype(bf16)
    biasm = conv_b.reshape(4, 128).T.astype(np.float32)
    idsb = input_ids.astype(np.float32).astype(bf16)

    in_maps = [{"ids": np.ascontiguousarray(idsb[c:c + 1]), "gws": gws, "iot": iot,
                "scw": scw, "bias": biasm} for c in range(B)]
    res = run_bass_kernel_spmd(nc, in_maps, core_ids=list(range(B)), trace=TRACE)
    LAST_RESULT = res
    return np.stack([np.asarray(res.results[c]["out"]).astype(np.float32)
                     for c in range(B)])


# revision 4
# speedup vs baseline: 1.4979x; 1.0578x over previous
"""GBST embedding kernel for Trainium2, data-parallel over batch on 8 cores.

v2: pipelined per-segment structure.
- Conv folded with embedding gather via onehot matmul (vocab contraction),
  y stored in even/odd deinterleaved planes (unit-stride DVE combine).
- b2+b4 combine terms folded into per-t row weights we/wo (one product per
  y plane); b4 residue handled via a reversed-pair strided view of p2.
- b3 handled on the u-grid with 4 residue-class rows, products unit-stride.
- Softmax per 2048-l segment in l-major [128,16] layout; s3 pooling done
  multi-partition via a [114,18] reshape trick.
- Output written bf16 via DMA xbar transpose (no PE transposes); host
  converts to f32.
"""
import sys
sys.path.insert(0, "/opt/trn_rl_repo")
import numpy as np
import ml_dtypes

import concourse.bass as bass
import concourse.bacc as bacc
import concourse.tile as tile
from concourse import mybir
from concourse.bass_utils import run_bass_kernel_spmd

bf16 = ml_dtypes.bfloat16
F32 = mybir.dt.float32
BF = mybir.dt.bfloat16
OP = mybir.AluOpType

L, T, V, D, K = 8192, 4096, 256, 512, 5
NDC, NVC, NLT, LTS = 4, 2, 16, 512
NSEG, SEGT, SEGL = 4, 1024, 2048
HP = 4104          # per-parity plane width (4096 + 8 pad)

TRACE = False
LAST_RESULT = None
_NC = None


def _seg_windows(s):
    """Per-seg b3 class windows on the m-grid (t=3m/3m+1/3m+2)."""
    t0, t1 = s * SEGT, (s + 1) * SEGT
    mA0 = -(-t0 // 3)
    nA = len(range(3 * mA0, t1, 3))
    mB0 = -(-(t0 - 1) // 3)
    nB = len(range(3 * mB0 + 1, t1, 3))
    mD0 = -(-(t0 - 2) // 3)
    nD = len(range(3 * mD0 + 2, t1, 3))
    mJ0 = min(mA0, mB0, mD0)
    mHI = max(mA0 + nA, mB0 + nB, mD0 + nD)
    return t0, mA0, nA, mB0, nB, mD0, nD, mJ0, mHI




def _sl(ap, start, n, step):
    """Tight strided slice of n elements from an AP's free dim."""
    return ap[:, start:start + step * (n - 1) + 1:step]


def _sl1(ap, start, n, step):
    return ap[0:1, start:start + step * (n - 1) + 1:step]


def _build():
    nc = bacc.Bacc("TRN2", target_bir_lowering=False)
    ids_d = nc.dram_tensor("ids", [1, L], BF, kind="ExternalInput")
    gws_d = nc.dram_tensor("gws", [128, 40 * 128], BF, kind="ExternalInput")
    iot_d = nc.dram_tensor("iot", [128, 2], F32, kind="ExternalInput")
    scw_d = nc.dram_tensor("scw", [128, 4], BF, kind="ExternalInput")
    bias_d = nc.dram_tensor("bias", [128, 4], F32, kind="ExternalInput")
    out_d = nc.dram_tensor("out", [T, D], BF, kind="ExternalOutput")
    # DRAM staging
    s1row_d = nc.dram_tensor("s1row_d", [1, L + 16], F32)
    us3_d = nc.dram_tensor("us3_d", [1, NSEG * 2052], F32)
    w3row_d = nc.dram_tensor("w3row_d", [1, L + 16], BF)
    we_d = nc.dram_tensor("we_d", [1, T], BF)
    wo_d = nc.dram_tensor("wo_d", [1, T], BF)
    cw4_d = nc.dram_tensor("cw4_d", [1, T], BF)
    cwrow_d = [nc.dram_tensor(f"cw{cl}_d", [1, NSEG * 348], BF)
               for cl in "abcd"]

    with tile.TileContext(nc) as tc:
        with tc.tile_pool(name="const", bufs=1) as cst, \
             tc.tile_pool(name="ydp", bufs=1) as ydp, \
             tc.tile_pool(name="ohp", bufs=2) as ohp, \
             tc.tile_pool(name="s1p", bufs=2) as s1p, \
             tc.tile_pool(name="smp", bufs=2) as smp, \
             tc.tile_pool(name="rwp", bufs=2) as rwp, \
             tc.tile_pool(name="wsg", bufs=2) as wsg, \
             tc.tile_pool(name="cmb", bufs=2) as cmb, \
             tc.tile_pool(name="accp", bufs=8) as accp, \
             tc.tile_pool(name="ctp", bufs=3) as ctp, \
             tc.tile_pool(name="otp", bufs=3) as otp, \
             tc.tile_pool(name="psA", bufs=3, space="PSUM") as psA, \
             tc.tile_pool(name="psB", bufs=2, space="PSUM") as psB:

            # ---- constants
            gws_t = cst.tile([128, 40 * 128], BF)
            nc.sync.dma_start(out=gws_t[:], in_=gws_d[:])
            iot_t = cst.tile([128, 2], F32)
            nc.sync.dma_start(out=iot_t[:], in_=iot_d[:])
            scw_t = cst.tile([128, 4], BF)
            nc.sync.dma_start(out=scw_t[:], in_=scw_d[:])
            bias_t = cst.tile([128, 4], F32)
            nc.sync.dma_start(out=bias_t[:], in_=bias_d[:])
            zpad_t = cst.tile([1, 16], F32)
            nc.vector.memset(zpad_t[:], 0.0)
            nc.sync.dma_start(out=s1row_d[0:1, L:L + 16], in_=zpad_t[:])

            # ---- persistent y planes: [128, 2, HP] (even | odd)
            yd = [ydp.tile([128, 2 * HP], BF, name=f"yd{dc}", tag=f"yd{dc}")
                  for dc in range(NDC)]
            ydv = [yd[dc][:].rearrange("p (h c) -> p h c", h=2)
                   for dc in range(NDC)]
            for dc in range(NDC):
                nc.vector.memset(ydv[dc][:, :, T:HP], 0.0)

            def conv_tile(i):
                c0 = i * LTS - 2
                c1 = i * LTS + 514
                lo = max(c0, 0)
                hi = min(c1, L)
                d0 = lo - c0
                d1 = 516 - (c1 - hi)
                idst = ohp.tile([128, 516], BF, tag="idst", name=f"idst{i}")
                nc.sync.dma_start(out=idst[:, d0:d1],
                                  in_=ids_d[0:1, lo:hi].partition_broadcast(128))
                ohs = []
                for vc in range(NVC):
                    oh = ohp.tile([128, 516], BF, tag=f"oh{vc}", name=f"oh{vc}_{i}")
                    if d0 > 0:
                        nc.vector.memset(oh[:, 0:d0], 0.0)
                    if d1 < 516:
                        nc.vector.memset(oh[:, d1:516], 0.0)
                    nc.vector.tensor_scalar(out=oh[:, d0:d1], in0=idst[:, d0:d1],
                                            scalar1=iot_t[:, vc:vc + 1], scalar2=None,
                                            op0=OP.is_equal)
                    ohs.append(oh)
                for dc in range(NDC):
                    ps = psA.tile([128, LTS], F32, tag="convps", name=f"ps_{i}_{dc}")
                    for j in range(10):
                        k, vc = divmod(j, 2)
                        nc.tensor.matmul(
                            out=ps[:],
                            lhsT=gws_t[:, ((k * 2 + vc) * 4 + dc) * 128:
                                       ((k * 2 + vc) * 4 + dc) * 128 + 128],
                            rhs=ohs[vc][:, k:k + LTS],
                            start=(j == 0), stop=(j == 9))
                    # PSUM -> SBUF deinterleaved into even/odd planes
                    nc.scalar.activation(
                        out=ydv[dc][:, :, i * 256:(i + 1) * 256],
                        in_=ps[:].rearrange("p (c h) -> p h c", h=2),
                        func=mybir.ActivationFunctionType.Identity,
                        bias=bias_t[:, dc:dc + 1])
                # s1 with parity-major psum ([s1e | s1o]); interleave on copy
                ps1 = psB.tile([1, LTS], F32, tag="s1ps", name=f"ps1_{i}")
                for dc in range(NDC):
                    nc.tensor.matmul(out=ps1[:], lhsT=scw_t[:, dc:dc + 1],
                                     rhs=ydv[dc][:, :, i * 256:(i + 1) * 256],
                                     start=(dc == 0), stop=(dc == NDC - 1))
                s1t = s1p.tile([1, LTS], F32, tag="s1t", name=f"s1t_{i}")
                nc.scalar.copy(out=s1t[:].rearrange("p (c h) -> p h c", h=2),
                               in_=ps1[:].rearrange("p (h c) -> p h c", h=2))
                nc.sync.dma_start(out=s1row_d[0:1, i * LTS:(i + 1) * LTS],
                                  in_=s1t[:])

            def scores_seg(s):
                t0 = s * SEGT
                l0 = 2 * t0
                j0 = l0 // 3
                r0 = l0 - 3 * j0
                # --- candidate scores in l-major [128, 16]
                S1 = smp.tile([128, 16], F32, tag="S1", name=f"S1_{s}")
                nc.sync.dma_start(out=S1[:], in_=s1row_d[0:1, l0:l0 + SEGL])
                Tt = smp.tile([114, 18], F32, tag="Tt", name=f"Tt_{s}")
                nc.sync.dma_start(out=Tt[:],
                                  in_=s1row_d[0:1, 3 * j0:3 * j0 + 2052])
                T3 = smp.tile([114, 6], F32, tag="T3", name=f"T3_{s}")
                Tv = Tt[:].rearrange("p (c three) -> p c three", three=3)
                nc.vector.tensor_tensor(out=T3[:], in0=Tv[:, :, 0], in1=Tv[:, :, 1],
                                        op=OP.add)
                nc.vector.tensor_tensor(out=T3[:], in0=T3[:], in1=Tv[:, :, 2],
                                        op=OP.add)
                U = smp.tile([114, 18], F32, tag="U", name=f"U_{s}")
                nc.vector.tensor_copy(
                    out=U[:].rearrange("p (c three) -> p c three", three=3),
                    in_=T3[:].unsqueeze(2).to_broadcast([114, 6, 3]))
                nc.sync.dma_start(out=us3_d[0:1, s * 2052:(s + 1) * 2052], in_=U[:])
                S3c = smp.tile([128, 16], F32, tag="S3c", name=f"S3c_{s}")
                nc.sync.dma_start(out=S3c[:],
                                  in_=us3_d[0:1, s * 2052 + r0:s * 2052 + r0 + SEGL])
                s2 = smp.tile([128, 8], F32, tag="s2", name=f"s2_{s}")
                S1pair = S1[:].rearrange("p (n two) -> p n two", two=2)
                nc.vector.tensor_tensor(out=s2[:], in0=S1pair[:, :, 0],
                                        in1=S1pair[:, :, 1], op=OP.add)
                s4 = smp.tile([128, 4], F32, tag="s4", name=f"s4_{s}")
                s2pair = s2[:].rearrange("p (n two) -> p n two", two=2)
                nc.vector.tensor_tensor(out=s4[:], in0=s2pair[:, :, 0],
                                        in1=s2pair[:, :, 1], op=OP.add)
                s2h = smp.tile([128, 8], F32, tag="s2h", name=f"s2h_{s}")
                nc.vector.tensor_scalar(out=s2h[:], in0=s2[:], scalar1=0.5,
                                        scalar2=None, op0=OP.mult)
                s4q = smp.tile([128, 4], F32, tag="s4q", name=f"s4q_{s}")
                nc.vector.tensor_scalar(out=s4q[:], in0=s4[:], scalar1=0.25,
                                        scalar2=None, op0=OP.mult)
                S3s = smp.tile([128, 16], F32, tag="S3s", name=f"S3s_{s}")
                nc.vector.tensor_scalar(out=S3s[:], in0=S3c[:], scalar1=1.0 / 3.0,
                                        scalar2=None, op0=OP.mult)
                # --- softmax
                mM = smp.tile([128, 16], F32, tag="mM", name=f"mM_{s}")
                nc.vector.tensor_tensor(out=mM[:], in0=S1[:], in1=S3s[:], op=OP.max)
                nc.vector.tensor_tensor(
                    out=mM[:].rearrange("p (n two) -> p n two", two=2),
                    in0=mM[:].rearrange("p (n two) -> p n two", two=2),
                    in1=s2h[:].unsqueeze(2).to_broadcast([128, 8, 2]), op=OP.max)
                nc.vector.tensor_tensor(
                    out=mM[:].rearrange("p (n four) -> p n four", four=4),
                    in0=mM[:].rearrange("p (n four) -> p n four", four=4),
                    in1=s4q[:].unsqueeze(2).to_broadcast([128, 4, 4]), op=OP.max)
                S = smp.tile([128, 64], F32, tag="S", name=f"S_{s}")
                nc.vector.tensor_tensor(out=S[:, 0:16], in0=S1[:], in1=mM[:],
                                        op=OP.subtract)
                nc.vector.tensor_tensor(
                    out=S[:, 16:32].rearrange("p (n two) -> p n two", two=2),
                    in0=s2h[:].unsqueeze(2).to_broadcast([128, 8, 2]),
                    in1=mM[:].rearrange("p (n two) -> p n two", two=2),
                    op=OP.subtract)
                nc.vector.tensor_tensor(out=S[:, 32:48], in0=S3s[:], in1=mM[:],
                                        op=OP.subtract)
                nc.vector.tensor_tensor(
                    out=S[:, 48:64].rearrange("p (n four) -> p n four", four=4),
                    in0=s4q[:].unsqueeze(2).to_broadcast([128, 4, 4]),
                    in1=mM[:].rearrange("p (n four) -> p n four", four=4),
                    op=OP.subtract)
                nc.scalar.activation(out=S[:], in_=S[:],
                                     func=mybir.ActivationFunctionType.Exp)
                Z = smp.tile([128, 16], F32, tag="Z", name=f"Z_{s}")
                S4v = S[:].rearrange("p (four n) -> p four n", four=4)
                nc.vector.tensor_tensor(out=Z[:], in0=S4v[:, 0], in1=S4v[:, 1],
                                        op=OP.add)
                nc.vector.tensor_tensor(out=Z[:], in0=Z[:], in1=S4v[:, 2], op=OP.add)
                nc.vector.tensor_tensor(out=Z[:], in0=Z[:], in1=S4v[:, 3], op=OP.add)
                R = smp.tile([128, 16], F32, tag="R", name=f"R_{s}")
                nc.vector.reciprocal(out=R[:], in_=Z[:])
                W = smp.tile([128, 64], F32, tag="W", name=f"W_{s}")
                nc.vector.tensor_tensor(
                    out=W[:].rearrange("p (four n) -> p four n", four=4), in0=S4v,
                    in1=R[:].unsqueeze(1).to_broadcast([128, 4, 16]), op=OP.mult)
                # --- weight rows
                W1 = W[:, 0:16].rearrange("p (n two) -> p n two", two=2)
                W2 = W[:, 16:32].rearrange("p (n two) -> p n two", two=2)
                W4 = W[:, 48:64].rearrange("p (n two) -> p n two", two=2)
                c2 = smp.tile([128, 8], F32, tag="c2", name=f"c2_{s}")
                nc.vector.tensor_tensor(out=c2[:], in0=W2[:, :, 0], in1=W2[:, :, 1],
                                        op=OP.add)
                c4 = smp.tile([128, 8], F32, tag="c4", name=f"c4_{s}")
                nc.vector.tensor_tensor(out=c4[:], in0=W4[:, :, 0], in1=W4[:, :, 1],
                                        op=OP.add)
                c4s = smp.tile([128, 8], F32, tag="c4s", name=f"c4s_{s}")
                nc.vector.tensor_scalar(out=c4s[:], in0=c4[:], scalar1=0.125,
                                        scalar2=None, op0=OP.mult)
                c24 = smp.tile([128, 8], F32, tag="c24", name=f"c24_{s}")
                nc.vector.scalar_tensor_tensor(out=c24[:], in0=c2[:], scalar=0.25,
                                               in1=c4s[:], op0=OP.mult, op1=OP.add)
                we_t = smp.tile([128, 8], BF, tag="we_t", name=f"we_t_{s}")
                nc.vector.scalar_tensor_tensor(out=we_t[:], in0=W1[:, :, 0],
                                               scalar=0.5, in1=c24[:],
                                               op0=OP.mult, op1=OP.add)
                wo_t = smp.tile([128, 8], BF, tag="wo_t", name=f"wo_t_{s}")
                nc.vector.scalar_tensor_tensor(out=wo_t[:], in0=W1[:, :, 1],
                                               scalar=0.5, in1=c24[:],
                                               op0=OP.mult, op1=OP.add)
                cw4_t = smp.tile([128, 8], BF, tag="cw4_t", name=f"cw4_t_{s}")
                nc.vector.tensor_copy(out=cw4_t[:], in_=c4s[:])
                w3w = smp.tile([128, 16], BF, tag="w3w", name=f"w3w_{s}")
                nc.vector.tensor_scalar(out=w3w[:], in0=W[:, 32:48],
                                        scalar1=1.0 / 6.0, scalar2=None, op0=OP.mult)
                nc.sync.dma_start(out=we_d[0:1, t0:t0 + SEGT], in_=we_t[:])
                nc.sync.dma_start(out=wo_d[0:1, t0:t0 + SEGT], in_=wo_t[:])
                nc.sync.dma_start(out=cw4_d[0:1, t0:t0 + SEGT], in_=cw4_t[:])
                nc.sync.dma_start(out=w3row_d[0:1, l0:l0 + SEGL], in_=w3w[:])
                # --- b3 class rows from w3
                t0_, mA0, nA, mB0, nB, mD0, nD, mJ0, mHI = _seg_windows(s)
                wlo = max(0, l0 - 4)
                w3seg = rwp.tile([1, 2064], BF, tag="w3seg", name=f"w3seg_{s}")
                nc.sync.dma_start(out=w3seg[0:1, 0:l0 + SEGL - wlo],
                                  in_=w3row_d[0:1, wlo:l0 + SEGL])
                cwa_t = rwp.tile([1, 348], BF, tag="cwa_t", name=f"cwa_t_{s}")
                a0 = 6 * mA0 - wlo
                nc.vector.tensor_tensor(out=cwa_t[0:1, 0:nA],
                                        in0=_sl1(w3seg, a0, nA, 6),
                                        in1=_sl1(w3seg, a0 + 1, nA, 6),
                                        op=OP.add)
                cwb_t = rwp.tile([1, 348], BF, tag="cwb_t", name=f"cwb_t_{s}")
                b0 = 6 * mB0 + 2 - wlo
                nc.vector.tensor_copy(out=cwb_t[0:1, 0:nB],
                                      in_=_sl1(w3seg, b0, nB, 6))
                cwc_t = rwp.tile([1, 348], BF, tag="cwc_t", name=f"cwc_t_{s}")
                nc.vector.tensor_copy(out=cwc_t[0:1, 0:nB],
                                      in_=_sl1(w3seg, b0 + 1, nB, 6))
                cwd_t = rwp.tile([1, 348], BF, tag="cwd_t", name=f"cwd_t_{s}")
                d0_ = 6 * mD0 + 4 - wlo
                nc.vector.tensor_tensor(out=cwd_t[0:1, 0:nD],
                                        in0=_sl1(w3seg, d0_, nD, 6),
                                        in1=_sl1(w3seg, d0_ + 1, nD, 6),
                                        op=OP.add)
                for cl, tl, n in ((0, cwa_t, nA), (1, cwb_t, nB), (2, cwc_t, nB),
                                  (3, cwd_t, nD)):
                    nc.sync.dma_start(out=cwrow_d[cl][0:1, s * 348:s * 348 + n],
                                      in_=tl[0:1, 0:n])

            ov = out_d[:].rearrange("(tb p) (dc c) -> p tb dc c", p=128, c=128)
            pend = {}

            def flush_seg(s):
                for dc in range(NDC):
                    acc = pend.pop((s, dc))
                    otr = otp.tile([128, 8 * 128], BF, tag="otr",
                                   name=f"otr_{s}_{dc}")
                    nc.sync.dma_start_transpose(
                        out=otr[:].rearrange("p (tb c) -> p tb c", c=128),
                        in_=acc[:])
                    nc.sync.dma_start(
                        out=ov[:, s * 8:(s + 1) * 8, dc, :],
                        in_=otr[:].rearrange("p (tb c) -> p tb c", c=128))

            def combine_seg(s):
                t0, mA0, nA, mB0, nB, mD0, nD, mJ0, mHI = _seg_windows(s)
                nJ = mHI - mJ0
                we_s = wsg.tile([128, SEGT], BF, tag="we_s", name=f"we_s_{s}")
                nc.sync.dma_start(
                    out=we_s[:], in_=we_d[0:1, t0:t0 + SEGT].partition_broadcast(128))
                wo_s = wsg.tile([128, SEGT], BF, tag="wo_s", name=f"wo_s_{s}")
                nc.sync.dma_start(
                    out=wo_s[:], in_=wo_d[0:1, t0:t0 + SEGT].partition_broadcast(128))
                cw4_s = wsg.tile([128, SEGT], BF, tag="cw4_s", name=f"cw4_s_{s}")
                nc.sync.dma_start(
                    out=cw4_s[:],
                    in_=cw4_d[0:1, t0:t0 + SEGT].partition_broadcast(128))
                cw_s = []
                for cl, n in ((0, nA), (1, nB), (2, nB), (3, nD)):
                    t_ = wsg.tile([128, 348], BF, tag=f"cw{cl}_s",
                                  name=f"cw{cl}_s_{s}")
                    nc.sync.dma_start(
                        out=t_[:, 0:n],
                        in_=cwrow_d[cl][0:1, s * 348:s * 348 + n]
                        .partition_broadcast(128))
                    cw_s.append(t_)

                for dc in range(NDC):
                    ye = ydv[dc][:, 0, :]
                    yo = ydv[dc][:, 1, :]
                    # p2 over [t0-2, t0+1026)
                    p2 = cmb.tile([128, 1028], BF, tag="p2", name=f"p2_{s}_{dc}")
                    if s == 0:
                        nc.vector.memset(p2[:, 0:2], 0.0)
                        nc.vector.tensor_tensor(out=p2[:, 2:1028],
                                                in0=ye[:, 0:1026],
                                                in1=yo[:, 0:1026], op=OP.add)
                    else:
                        nc.vector.tensor_tensor(out=p2[:, 0:1028],
                                                in0=ye[:, t0 - 2:t0 + 1026],
                                                in1=yo[:, t0 - 2:t0 + 1026],
                                                op=OP.add)
                    m_e = ctp.tile([128, SEGT], BF, tag="ct", name=f"me_{s}_{dc}")
                    nc.vector.tensor_tensor(out=m_e[:], in0=ye[:, t0:t0 + SEGT],
                                            in1=we_s[:], op=OP.mult)
                    m_o = ctp.tile([128, SEGT], BF, tag="ct", name=f"mo_{s}_{dc}")
                    nc.vector.tensor_tensor(out=m_o[:], in0=yo[:, t0:t0 + SEGT],
                                            in1=wo_s[:], op=OP.mult)
                    acc = accp.tile([128, SEGT], BF, tag="acc",
                                    name=f"acc_{s}_{dc}")
                    nc.vector.tensor_tensor(out=acc[:], in0=m_e[:], in1=m_o[:],
                                            op=OP.add)
                    # b4 residue: cw4[t] * p2[swap(t)]
                    s4t = ctp.tile([128, SEGT], BF, tag="ct", name=f"s4t_{s}_{dc}")
                    p2sw = p2[:, 2:1026].rearrange("p (v two) -> p v two",
                                                   two=2)[:, :, ::-1]
                    nc.vector.tensor_tensor(
                        out=s4t[:].rearrange("p (v two) -> p v two", two=2),
                        in0=p2sw,
                        in1=cw4_s[:].rearrange("p (v two) -> p v two", two=2),
                        op=OP.mult)
                    nc.vector.tensor_tensor(out=acc[:], in0=acc[:], in1=s4t[:],
                                            op=OP.add)
                    # b3 pooled planes on the m-grid [mJ0, mHI)
                    p3e = cmb.tile([128, 348], BF, tag="p3e", name=f"p3e_{s}_{dc}")
                    e0 = 3 * mJ0 - (t0 - 2)
                    nc.vector.tensor_tensor(
                        out=p3e[:, 0:nJ],
                        in0=_sl(p2, e0, nJ, 3),
                        in1=_sl(ye, 3 * mJ0 + 1, nJ, 3), op=OP.add)
                    p3o = cmb.tile([128, 348], BF, tag="p3o", name=f"p3o_{s}_{dc}")
                    nc.vector.tensor_tensor(
                        out=p3o[:, 0:nJ],
                        in0=_sl(yo, 3 * mJ0 + 1, nJ, 3),
                        in1=_sl(p2, e0 + 2, nJ, 3), op=OP.add)
                    tb = []
                    for bi, (p3t, m0, n, cwt) in enumerate((
                            (p3e, mA0, nA, cw_s[0]), (p3e, mB0, nB, cw_s[1]),
                            (p3o, mB0, nB, cw_s[2]), (p3o, mD0, nD, cw_s[3]))):
                        t_ = otp.tile([128, 348], BF, tag="tb",
                                      name=f"tb{bi}_{s}_{dc}")
                        o = m0 - mJ0
                        nc.vector.tensor_tensor(out=t_[:, 0:n],
                                                in0=p3t[:, o:o + n],
                                                in1=cwt[:, 0:n], op=OP.mult)
                        tb.append(t_)
                    X = otp.tile([128, 348], BF, tag="tb", name=f"X_{s}_{dc}")
                    nc.vector.tensor_tensor(out=X[:, 0:nB], in0=tb[1][:, 0:nB],
                                            in1=tb[2][:, 0:nB], op=OP.add)
                    for (t_, m0, n, coff) in ((tb[0], mA0, nA, 0), (X, mB0, nB, 1),
                                              (tb[3], mD0, nD, 2)):
                        st = 3 * m0 + coff - t0
                        av = _sl(acc, st, n, 3)
                        nc.vector.tensor_tensor(out=av, in0=av, in1=t_[:, 0:n],
                                                op=OP.add)
                    pend[(s, dc)] = acc

            # ---- emission: conv group precedes its segment's scores so the
            # DVE stream never blocks conv onehots behind a waiting scores op
            for i in range(5):
                conv_tile(i)
            for s in range(NSEG):
                for i in range(4 * s + 5, min(4 * s + 9, NLT)):
                    conv_tile(i)
                scores_seg(s)
                if s >= 1:
                    flush_seg(s - 1)
                combine_seg(s)
            flush_seg(NSEG - 1)
    nc.compile()
    return nc


def _get_nc():
    global _NC
    if _NC is None:
        _NC = _build()
    return _NC


def kernel(input_ids, emb, conv_w, conv_b, score_w):
    global LAST_RESULT
    nc = _get_nc()
    input_ids = np.asarray(input_ids)
    emb = np.asarray(emb, dtype=np.float32)
    conv_w = np.asarray(conv_w, dtype=np.float32)
    conv_b = np.asarray(conv_b, dtype=np.float32)
    score_w = np.asarray(score_w, dtype=np.float32)
    B = input_ids.shape[0]

    G = np.einsum("oik,vi->kvo", conv_w.astype(np.float64),
                  emb.astype(np.float64)).astype(np.float32)  # [K, V, D]
    gws = np.zeros((128, 40, 128), np.float32)
    for k in range(K):
        for vc in range(NVC):
            for dc in range(NDC):
                gws[:, (k * 2 + vc) * 4 + dc, :] = \
                    G[k, vc * 128:(vc + 1) * 128, dc * 128:(dc + 1) * 128]
    gws = gws.reshape(128, 40 * 128).astype(bf16)
    iot = np.stack([np.arange(128), np.arange(128) + 128], axis=1).astype(np.float32)
    scw = score_w.reshape(4, 128).T.astype(bf16)
    biasm = conv_b.reshape(4, 128).T.astype(np.float32)
    idsb = input_ids.astype(np.float32).astype(bf16)

    in_maps = [{"ids": np.ascontiguousarray(idsb[c:c + 1]), "gws": gws, "iot": iot,
                "scw": scw, "bias": biasm} for c in range(B)]
    res = run_bass_kernel_spmd(nc, in_maps, core_ids=list(range(B)), trace=TRACE)
    LAST_RESULT = res
    return np.stack([np.asarray(res.results[c]["out"]).astype(np.float32)
                     for c in range(B)])


# revision 5
# speedup vs baseline: 1.5132x; 1.0102x over previous
"""GBST embedding kernel for Trainium2, data-parallel over batch on 8 cores.

v2: pipelined per-segment structure.
- Conv folded with embedding gather via onehot matmul (vocab contraction),
  y stored in even/odd deinterleaved planes (unit-stride DVE combine).
- b2+b4 combine terms folded into per-t row weights we/wo (one product per
  y plane); b4 residue handled via a reversed-pair strided view of p2.
- b3 handled on the u-grid with 4 residue-class rows, products unit-stride.
- Softmax per 2048-l segment in l-major [128,16] layout; s3 pooling done
  multi-partition via a [114,18] reshape trick.
- Output written bf16 via DMA xbar transpose (no PE transposes); host
  converts to f32.
"""
import sys
sys.path.insert(0, "/opt/trn_rl_repo")
import numpy as np
import ml_dtypes

import concourse.bass as bass
import concourse.bacc as bacc
import concourse.tile as tile
from concourse import mybir
from concourse.bass_utils import run_bass_kernel_spmd

bf16 = ml_dtypes.bfloat16
F32 = mybir.dt.float32
BF = mybir.dt.bfloat16
OP = mybir.AluOpType

L, T, V, D, K = 8192, 4096, 256, 512, 5
NDC, NVC, NLT, LTS = 4, 2, 16, 512
NSEG, SEGT, SEGL = 4, 1024, 2048
HP = 4104          # per-parity plane width (4096 + 8 pad)

TRACE = False
LAST_RESULT = None
_NC = None


def _seg_windows(s):
    """Per-seg b3 class windows on the m-grid (t=3m/3m+1/3m+2)."""
    t0, t1 = s * SEGT, (s + 1) * SEGT
    mA0 = -(-t0 // 3)
    nA = len(range(3 * mA0, t1, 3))
    mB0 = -(-(t0 - 1) // 3)
    nB = len(range(3 * mB0 + 1, t1, 3))
    mD0 = -(-(t0 - 2) // 3)
    nD = len(range(3 * mD0 + 2, t1, 3))
    mJ0 = min(mA0, mB0, mD0)
    mHI = max(mA0 + nA, mB0 + nB, mD0 + nD)
    return t0, mA0, nA, mB0, nB, mD0, nD, mJ0, mHI




def _sl(ap, start, n, step):
    """Tight strided slice of n elements from an AP's free dim."""
    return ap[:, start:start + step * (n - 1) + 1:step]


def _sl1(ap, start, n, step):
    return ap[0:1, start:start + step * (n - 1) + 1:step]


def _build():
    nc = bacc.Bacc("TRN2", target_bir_lowering=False)
    ids_d = nc.dram_tensor("ids", [1, L], BF, kind="ExternalInput")
    gws_d = nc.dram_tensor("gws", [128, 40 * 128], BF, kind="ExternalInput")
    iot_d = nc.dram_tensor("iot", [128, 2], F32, kind="ExternalInput")
    scw_d = nc.dram_tensor("scw", [128, 4], BF, kind="ExternalInput")
    bias_d = nc.dram_tensor("bias", [128, 4], F32, kind="ExternalInput")
    out_d = nc.dram_tensor("out", [T, D], BF, kind="ExternalOutput")
    # DRAM staging
    s1row_d = nc.dram_tensor("s1row_d", [1, L + 16], F32)
    us3_d = nc.dram_tensor("us3_d", [1, NSEG * 2052], F32)
    w3row_d = nc.dram_tensor("w3row_d", [1, L + 16], BF)
    we_d = nc.dram_tensor("we_d", [1, T], BF)
    wo_d = nc.dram_tensor("wo_d", [1, T], BF)
    cw4_d = nc.dram_tensor("cw4_d", [1, T], BF)
    cwrow_d = [nc.dram_tensor(f"cw{cl}_d", [1, NSEG * 348], BF)
               for cl in "abcd"]

    with tile.TileContext(nc) as tc:
        with tc.tile_pool(name="const", bufs=1) as cst, \
             tc.tile_pool(name="ydp", bufs=1) as ydp, \
             tc.tile_pool(name="ohp", bufs=4) as ohp, \
             tc.tile_pool(name="s1p", bufs=2) as s1p, \
             tc.tile_pool(name="smp", bufs=2) as smp, \
             tc.tile_pool(name="rwp", bufs=2) as rwp, \
             tc.tile_pool(name="wsg", bufs=3) as wsg, \
             tc.tile_pool(name="cmb", bufs=2) as cmb, \
             tc.tile_pool(name="accp", bufs=8) as accp, \
             tc.tile_pool(name="ctp", bufs=4) as ctp, \
             tc.tile_pool(name="otp", bufs=3) as otp, \
             tc.tile_pool(name="psA", bufs=4, space="PSUM") as psA, \
             tc.tile_pool(name="psB", bufs=2, space="PSUM") as psB:

            # ---- constants
            gws_t = cst.tile([128, 40 * 128], BF)
            nc.sync.dma_start(out=gws_t[:], in_=gws_d[:])
            iot_t = cst.tile([128, 2], F32)
            nc.sync.dma_start(out=iot_t[:], in_=iot_d[:])
            scw_t = cst.tile([128, 4], BF)
            nc.sync.dma_start(out=scw_t[:], in_=scw_d[:])
            bias_t = cst.tile([128, 4], F32)
            nc.sync.dma_start(out=bias_t[:], in_=bias_d[:])
            zpad_t = cst.tile([1, 16], F32)
            nc.vector.memset(zpad_t[:], 0.0)
            nc.sync.dma_start(out=s1row_d[0:1, L:L + 16], in_=zpad_t[:])

            # ---- persistent y planes: [128, 2, HP] (even | odd)
            yd = [ydp.tile([128, 2 * HP], BF, name=f"yd{dc}", tag=f"yd{dc}")
                  for dc in range(NDC)]
            ydv = [yd[dc][:].rearrange("p (h c) -> p h c", h=2)
                   for dc in range(NDC)]
            for dc in range(NDC):
                nc.vector.memset(ydv[dc][:, :, T:HP], 0.0)

            def conv_tile(i):
                c0 = i * LTS - 2
                c1 = i * LTS + 514
                lo = max(c0, 0)
                hi = min(c1, L)
                d0 = lo - c0
                d1 = 516 - (c1 - hi)
                idst = ohp.tile([128, 516], BF, tag="idst", name=f"idst{i}")
                nc.sync.dma_start(out=idst[:, d0:d1],
                                  in_=ids_d[0:1, lo:hi].partition_broadcast(128))
                ohs = []
                for vc in range(NVC):
                    oh = ohp.tile([128, 516], BF, tag=f"oh{vc}", name=f"oh{vc}_{i}")
                    if d0 > 0:
                        nc.vector.memset(oh[:, 0:d0], 0.0)
                    if d1 < 516:
                        nc.vector.memset(oh[:, d1:516], 0.0)
                    nc.vector.tensor_scalar(out=oh[:, d0:d1], in0=idst[:, d0:d1],
                                            scalar1=iot_t[:, vc:vc + 1], scalar2=None,
                                            op0=OP.is_equal)
                    ohs.append(oh)
                for dc in range(NDC):
                    ps = psA.tile([128, LTS], F32, tag="convps", name=f"ps_{i}_{dc}")
                    for j in range(10):
                        k, vc = divmod(j, 2)
                        nc.tensor.matmul(
                            out=ps[:],
                            lhsT=gws_t[:, ((k * 2 + vc) * 4 + dc) * 128:
                                       ((k * 2 + vc) * 4 + dc) * 128 + 128],
                            rhs=ohs[vc][:, k:k + LTS],
                            start=(j == 0), stop=(j == 9))
                    # PSUM -> SBUF deinterleaved into even/odd planes
                    nc.scalar.activation(
                        out=ydv[dc][:, :, i * 256:(i + 1) * 256],
                        in_=ps[:].rearrange("p (c h) -> p h c", h=2),
                        func=mybir.ActivationFunctionType.Identity,
                        bias=bias_t[:, dc:dc + 1])
                # s1 with parity-major psum ([s1e | s1o]); interleave on copy
                ps1 = psB.tile([1, LTS], F32, tag="s1ps", name=f"ps1_{i}")
                for dc in range(NDC):
                    nc.tensor.matmul(out=ps1[:], lhsT=scw_t[:, dc:dc + 1],
                                     rhs=ydv[dc][:, :, i * 256:(i + 1) * 256],
                                     start=(dc == 0), stop=(dc == NDC - 1))
                s1t = s1p.tile([1, LTS], F32, tag="s1t", name=f"s1t_{i}")
                nc.scalar.copy(out=s1t[:].rearrange("p (c h) -> p h c", h=2),
                               in_=ps1[:].rearrange("p (h c) -> p h c", h=2))
                nc.sync.dma_start(out=s1row_d[0:1, i * LTS:(i + 1) * LTS],
                                  in_=s1t[:])

            def scores_seg(s):
                t0 = s * SEGT
                l0 = 2 * t0
                j0 = l0 // 3
                r0 = l0 - 3 * j0
                # --- candidate scores in l-major [128, 16]
                S1 = smp.tile([128, 16], F32, tag="S1", name=f"S1_{s}")
                nc.sync.dma_start(out=S1[:], in_=s1row_d[0:1, l0:l0 + SEGL])
                Tt = smp.tile([114, 18], F32, tag="Tt", name=f"Tt_{s}")
                nc.sync.dma_start(out=Tt[:],
                                  in_=s1row_d[0:1, 3 * j0:3 * j0 + 2052])
                T3 = smp.tile([114, 6], F32, tag="T3", name=f"T3_{s}")
                Tv = Tt[:].rearrange("p (c three) -> p c three", three=3)
                nc.vector.tensor_tensor(out=T3[:], in0=Tv[:, :, 0], in1=Tv[:, :, 1],
                                        op=OP.add)
                nc.vector.tensor_tensor(out=T3[:], in0=T3[:], in1=Tv[:, :, 2],
                                        op=OP.add)
                U = smp.tile([114, 18], F32, tag="U", name=f"U_{s}")
                nc.vector.tensor_copy(
                    out=U[:].rearrange("p (c three) -> p c three", three=3),
                    in_=T3[:].unsqueeze(2).to_broadcast([114, 6, 3]))
                nc.sync.dma_start(out=us3_d[0:1, s * 2052:(s + 1) * 2052], in_=U[:])
                S3c = smp.tile([128, 16], F32, tag="S3c", name=f"S3c_{s}")
                nc.sync.dma_start(out=S3c[:],
                                  in_=us3_d[0:1, s * 2052 + r0:s * 2052 + r0 + SEGL])
                s2 = smp.tile([128, 8], F32, tag="s2", name=f"s2_{s}")
                S1pair = S1[:].rearrange("p (n two) -> p n two", two=2)
                nc.vector.tensor_tensor(out=s2[:], in0=S1pair[:, :, 0],
                                        in1=S1pair[:, :, 1], op=OP.add)
                s4 = smp.tile([128, 4], F32, tag="s4", name=f"s4_{s}")
                s2pair = s2[:].rearrange("p (n two) -> p n two", two=2)
                nc.vector.tensor_tensor(out=s4[:], in0=s2pair[:, :, 0],
                                        in1=s2pair[:, :, 1], op=OP.add)
                s2h = smp.tile([128, 8], F32, tag="s2h", name=f"s2h_{s}")
                nc.vector.tensor_scalar(out=s2h[:], in0=s2[:], scalar1=0.5,
                                        scalar2=None, op0=OP.mult)
                s4q = smp.tile([128, 4], F32, tag="s4q", name=f"s4q_{s}")
                nc.vector.tensor_scalar(out=s4q[:], in0=s4[:], scalar1=0.25,
                                        scalar2=None, op0=OP.mult)
                S3s = smp.tile([128, 16], F32, tag="S3s", name=f"S3s_{s}")
                nc.vector.tensor_scalar(out=S3s[:], in0=S3c[:], scalar1=1.0 / 3.0,
                                        scalar2=None, op0=OP.mult)
                # --- softmax
                mM = smp.tile([128, 16], F32, tag="mM", name=f"mM_{s}")
                nc.vector.tensor_tensor(out=mM[:], in0=S1[:], in1=S3s[:], op=OP.max)
                nc.vector.tensor_tensor(
                    out=mM[:].rearrange("p (n two) -> p n two", two=2),
                    in0=mM[:].rearrange("p (n two) -> p n two", two=2),
                    in1=s2h[:].unsqueeze(2).to_broadcast([128, 8, 2]), op=OP.max)
                nc.vector.tensor_tensor(
                    out=mM[:].rearrange("p (n four) -> p n four", four=4),
                    in0=mM[:].rearrange("p (n four) -> p n four", four=4),
                    in1=s4q[:].unsqueeze(2).to_broadcast([128, 4, 4]), op=OP.max)
                S = smp.tile([128, 64], F32, tag="S", name=f"S_{s}")
                nc.vector.tensor_tensor(out=S[:, 0:16], in0=S1[:], in1=mM[:],
                                        op=OP.subtract)
                nc.vector.tensor_tensor(
                    out=S[:, 16:32].rearrange("p (n two) -> p n two", two=2),
                    in0=s2h[:].unsqueeze(2).to_broadcast([128, 8, 2]),
                    in1=mM[:].rearrange("p (n two) -> p n two", two=2),
                    op=OP.subtract)
                nc.vector.tensor_tensor(out=S[:, 32:48], in0=S3s[:], in1=mM[:],
                                        op=OP.subtract)
                nc.vector.tensor_tensor(
                    out=S[:, 48:64].rearrange("p (n four) -> p n four", four=4),
                    in0=s4q[:].unsqueeze(2).to_broadcast([128, 4, 4]),
                    in1=mM[:].rearrange("p (n four) -> p n four", four=4),
                    op=OP.subtract)
                nc.scalar.activation(out=S[:], in_=S[:],
                                     func=mybir.ActivationFunctionType.Exp)
                Z = smp.tile([128, 16], F32, tag="Z", name=f"Z_{s}")
                S4v = S[:].rearrange("p (four n) -> p four n", four=4)
                nc.vector.tensor_tensor(out=Z[:], in0=S4v[:, 0], in1=S4v[:, 1],
                                        op=OP.add)
                nc.vector.tensor_tensor(out=Z[:], in0=Z[:], in1=S4v[:, 2], op=OP.add)
                nc.vector.tensor_tensor(out=Z[:], in0=Z[:], in1=S4v[:, 3], op=OP.add)
                R = smp.tile([128, 16], F32, tag="R", name=f"R_{s}")
                nc.vector.reciprocal(out=R[:], in_=Z[:])
                W = smp.tile([128, 64], F32, tag="W", name=f"W_{s}")
                nc.vector.tensor_tensor(
                    out=W[:].rearrange("p (four n) -> p four n", four=4), in0=S4v,
                    in1=R[:].unsqueeze(1).to_broadcast([128, 4, 16]), op=OP.mult)
                # --- weight rows
                W1 = W[:, 0:16].rearrange("p (n two) -> p n two", two=2)
                W2 = W[:, 16:32].rearrange("p (n two) -> p n two", two=2)
                W4 = W[:, 48:64].rearrange("p (n two) -> p n two", two=2)
                c2 = smp.tile([128, 8], F32, tag="c2", name=f"c2_{s}")
                nc.vector.tensor_tensor(out=c2[:], in0=W2[:, :, 0], in1=W2[:, :, 1],
                                        op=OP.add)
                c4 = smp.tile([128, 8], F32, tag="c4", name=f"c4_{s}")
                nc.vector.tensor_tensor(out=c4[:], in0=W4[:, :, 0], in1=W4[:, :, 1],
                                        op=OP.add)
                c4s = smp.tile([128, 8], F32, tag="c4s", name=f"c4s_{s}")
                nc.vector.tensor_scalar(out=c4s[:], in0=c4[:], scalar1=0.125,
                                        scalar2=None, op0=OP.mult)
                c24 = smp.tile([128, 8], F32, tag="c24", name=f"c24_{s}")
                nc.vector.scalar_tensor_tensor(out=c24[:], in0=c2[:], scalar=0.25,
                                               in1=c4s[:], op0=OP.mult, op1=OP.add)
                we_t = smp.tile([128, 8], BF, tag="we_t", name=f"we_t_{s}")
                nc.vector.scalar_tensor_tensor(out=we_t[:], in0=W1[:, :, 0],
                                               scalar=0.5, in1=c24[:],
                                               op0=OP.mult, op1=OP.add)
                wo_t = smp.tile([128, 8], BF, tag="wo_t", name=f"wo_t_{s}")
                nc.vector.scalar_tensor_tensor(out=wo_t[:], in0=W1[:, :, 1],
                                               scalar=0.5, in1=c24[:],
                                               op0=OP.mult, op1=OP.add)
                cw4_t = smp.tile([128, 8], BF, tag="cw4_t", name=f"cw4_t_{s}")
                nc.vector.tensor_copy(out=cw4_t[:], in_=c4s[:])
                w3w = smp.tile([128, 16], BF, tag="w3w", name=f"w3w_{s}")
                nc.vector.tensor_scalar(out=w3w[:], in0=W[:, 32:48],
                                        scalar1=1.0 / 6.0, scalar2=None, op0=OP.mult)
                nc.sync.dma_start(out=we_d[0:1, t0:t0 + SEGT], in_=we_t[:])
                nc.sync.dma_start(out=wo_d[0:1, t0:t0 + SEGT], in_=wo_t[:])
                nc.sync.dma_start(out=cw4_d[0:1, t0:t0 + SEGT], in_=cw4_t[:])
                nc.sync.dma_start(out=w3row_d[0:1, l0:l0 + SEGL], in_=w3w[:])
                # --- b3 class rows from w3
                t0_, mA0, nA, mB0, nB, mD0, nD, mJ0, mHI = _seg_windows(s)
                wlo = max(0, l0 - 4)
                w3seg = rwp.tile([1, 2064], BF, tag="w3seg", name=f"w3seg_{s}")
                nc.sync.dma_start(out=w3seg[0:1, 0:l0 + SEGL - wlo],
                                  in_=w3row_d[0:1, wlo:l0 + SEGL])
                cwa_t = rwp.tile([1, 348], BF, tag="cwa_t", name=f"cwa_t_{s}")
                a0 = 6 * mA0 - wlo
                nc.vector.tensor_tensor(out=cwa_t[0:1, 0:nA],
                                        in0=_sl1(w3seg, a0, nA, 6),
                                        in1=_sl1(w3seg, a0 + 1, nA, 6),
                                        op=OP.add)
                cwb_t = rwp.tile([1, 348], BF, tag="cwb_t", name=f"cwb_t_{s}")
                b0 = 6 * mB0 + 2 - wlo
                nc.vector.tensor_copy(out=cwb_t[0:1, 0:nB],
                                      in_=_sl1(w3seg, b0, nB, 6))
                cwc_t = rwp.tile([1, 348], BF, tag="cwc_t", name=f"cwc_t_{s}")
                nc.vector.tensor_copy(out=cwc_t[0:1, 0:nB],
                                      in_=_sl1(w3seg, b0 + 1, nB, 6))
                cwd_t = rwp.tile([1, 348], BF, tag="cwd_t", name=f"cwd_t_{s}")
                d0_ = 6 * mD0 + 4 - wlo
                nc.vector.tensor_tensor(out=cwd_t[0:1, 0:nD],
                                        in0=_sl1(w3seg, d0_, nD, 6),
                                        in1=_sl1(w3seg, d0_ + 1, nD, 6),
                                        op=OP.add)
                for cl, tl, n in ((0, cwa_t, nA), (1, cwb_t, nB), (2, cwc_t, nB),
                                  (3, cwd_t, nD)):
                    nc.sync.dma_start(out=cwrow_d[cl][0:1, s * 348:s * 348 + n],
                                      in_=tl[0:1, 0:n])

            ov = out_d[:].rearrange("(tb p) (dc c) -> p tb dc c", p=128, c=128)
            pend = {}

            def flush_seg(s):
                for dc in range(NDC):
                    acc = pend.pop((s, dc))
                    otr = otp.tile([128, 8 * 128], BF, tag="otr",
                                   name=f"otr_{s}_{dc}")
                    nc.sync.dma_start_transpose(
                        out=otr[:].rearrange("p (tb c) -> p tb c", c=128),
                        in_=acc[:])
                    nc.sync.dma_start(
                        out=ov[:, s * 8:(s + 1) * 8, dc, :],
                        in_=otr[:].rearrange("p (tb c) -> p tb c", c=128))

            def combine_seg(s):
                t0, mA0, nA, mB0, nB, mD0, nD, mJ0, mHI = _seg_windows(s)
                nJ = mHI - mJ0
                we_s = wsg.tile([128, SEGT], BF, tag="we_s", name=f"we_s_{s}")
                nc.sync.dma_start(
                    out=we_s[:], in_=we_d[0:1, t0:t0 + SEGT].partition_broadcast(128))
                wo_s = wsg.tile([128, SEGT], BF, tag="wo_s", name=f"wo_s_{s}")
                nc.sync.dma_start(
                    out=wo_s[:], in_=wo_d[0:1, t0:t0 + SEGT].partition_broadcast(128))
                cw4_s = wsg.tile([128, SEGT], BF, tag="cw4_s", name=f"cw4_s_{s}")
                nc.sync.dma_start(
                    out=cw4_s[:],
                    in_=cw4_d[0:1, t0:t0 + SEGT].partition_broadcast(128))
                cw_s = []
                for cl, n in ((0, nA), (1, nB), (2, nB), (3, nD)):
                    t_ = wsg.tile([128, 348], BF, tag=f"cw{cl}_s",
                                  name=f"cw{cl}_s_{s}")
                    nc.sync.dma_start(
                        out=t_[:, 0:n],
                        in_=cwrow_d[cl][0:1, s * 348:s * 348 + n]
                        .partition_broadcast(128))
                    cw_s.append(t_)

                for dc in range(NDC):
                    ye = ydv[dc][:, 0, :]
                    yo = ydv[dc][:, 1, :]
                    # p2 over [t0-2, t0+1026)
                    p2 = cmb.tile([128, 1028], BF, tag="p2", name=f"p2_{s}_{dc}")
                    if s == 0:
                        nc.vector.memset(p2[:, 0:2], 0.0)
                        nc.vector.tensor_tensor(out=p2[:, 2:1028],
                                                in0=ye[:, 0:1026],
                                                in1=yo[:, 0:1026], op=OP.add)
                    else:
                        nc.vector.tensor_tensor(out=p2[:, 0:1028],
                                                in0=ye[:, t0 - 2:t0 + 1026],
                                                in1=yo[:, t0 - 2:t0 + 1026],
                                                op=OP.add)
                    m_e = ctp.tile([128, SEGT], BF, tag="ct", name=f"me_{s}_{dc}")
                    nc.vector.tensor_tensor(out=m_e[:], in0=ye[:, t0:t0 + SEGT],
                                            in1=we_s[:], op=OP.mult)
                    m_o = ctp.tile([128, SEGT], BF, tag="ct", name=f"mo_{s}_{dc}")
                    nc.vector.tensor_tensor(out=m_o[:], in0=yo[:, t0:t0 + SEGT],
                                            in1=wo_s[:], op=OP.mult)
                    acc = accp.tile([128, SEGT], BF, tag="acc",
                                    name=f"acc_{s}_{dc}")
                    nc.vector.tensor_tensor(out=acc[:], in0=m_e[:], in1=m_o[:],
                                            op=OP.add)
                    # b4 residue: cw4[t] * p2[swap(t)]
                    s4t = ctp.tile([128, SEGT], BF, tag="ct", name=f"s4t_{s}_{dc}")
                    p2sw = p2[:, 2:1026].rearrange("p (v two) -> p v two",
                                                   two=2)[:, :, ::-1]
                    nc.vector.tensor_tensor(
                        out=s4t[:].rearrange("p (v two) -> p v two", two=2),
                        in0=p2sw,
                        in1=cw4_s[:].rearrange("p (v two) -> p v two", two=2),
                        op=OP.mult)
                    nc.vector.tensor_tensor(out=acc[:], in0=acc[:], in1=s4t[:],
                                            op=OP.add)
                    # b3 pooled planes on the m-grid [mJ0, mHI)
                    p3e = cmb.tile([128, 348], BF, tag="p3e", name=f"p3e_{s}_{dc}")
                    e0 = 3 * mJ0 - (t0 - 2)
                    nc.vector.tensor_tensor(
                        out=p3e[:, 0:nJ],
                        in0=_sl(p2, e0, nJ, 3),
                        in1=_sl(ye, 3 * mJ0 + 1, nJ, 3), op=OP.add)
                    p3o = cmb.tile([128, 348], BF, tag="p3o", name=f"p3o_{s}_{dc}")
                    nc.vector.tensor_tensor(
                        out=p3o[:, 0:nJ],
                        in0=_sl(yo, 3 * mJ0 + 1, nJ, 3),
                        in1=_sl(p2, e0 + 2, nJ, 3), op=OP.add)
                    tb = []
                    for bi, (p3t, m0, n, cwt) in enumerate((
                            (p3e, mA0, nA, cw_s[0]), (p3e, mB0, nB, cw_s[1]),
                            (p3o, mB0, nB, cw_s[2]), (p3o, mD0, nD, cw_s[3]))):
                        t_ = otp.tile([128, 348], BF, tag="tb",
                                      name=f"tb{bi}_{s}_{dc}")
                        o = m0 - mJ0
                        nc.vector.tensor_tensor(out=t_[:, 0:n],
                                                in0=p3t[:, o:o + n],
                                                in1=cwt[:, 0:n], op=OP.mult)
                        tb.append(t_)
                    X = otp.tile([128, 348], BF, tag="tb", name=f"X_{s}_{dc}")
                    nc.vector.tensor_tensor(out=X[:, 0:nB], in0=tb[1][:, 0:nB],
                                            in1=tb[2][:, 0:nB], op=OP.add)
                    for (t_, m0, n, coff) in ((tb[0], mA0, nA, 0), (X, mB0, nB, 1),
                                              (tb[3], mD0, nD, 2)):
                        st = 3 * m0 + coff - t0
                        av = _sl(acc, st, n, 3)
                        nc.vector.tensor_tensor(out=av, in0=av, in1=t_[:, 0:n],
                                                op=OP.add)
                    pend[(s, dc)] = acc

            # ---- emission: conv group precedes its segment's scores so the
            # DVE stream never blocks conv onehots behind a waiting scores op
            for i in range(5):
                conv_tile(i)
            for s in range(NSEG):
                for i in range(4 * s + 5, min(4 * s + 9, NLT)):
                    conv_tile(i)
                scores_seg(s)
                if s >= 1:
                    flush_seg(s - 1)
                combine_seg(s)
            flush_seg(NSEG - 1)
    nc.compile()
    return nc


def _get_nc():
    global _NC
    if _NC is None:
        _NC = _build()
    return _NC


def kernel(input_ids, emb, conv_w, conv_b, score_w):
    global LAST_RESULT
    nc = _get_nc()
    input_ids = np.asarray(input_ids)
    emb = np.asarray(emb, dtype=np.float32)
    conv_w = np.asarray(conv_w, dtype=np.float32)
    conv_b = np.asarray(conv_b, dtype=np.float32)
    score_w = np.asarray(score_w, dtype=np.float32)
    B = input_ids.shape[0]

    G = np.einsum("oik,vi->kvo", conv_w.astype(np.float64),
                  emb.astype(np.float64)).astype(np.float32)  # [K, V, D]
    gws = np.zeros((128, 40, 128), np.float32)
    for k in range(K):
        for vc in range(NVC):
            for dc in range(NDC):
                gws[:, (k * 2 + vc) * 4 + dc, :] = \
                    G[k, vc * 128:(vc + 1) * 128, dc * 128:(dc + 1) * 128]
    gws = gws.reshape(128, 40 * 128).astype(bf16)
    iot = np.stack([np.arange(128), np.arange(128) + 128], axis=1).astype(np.float32)
    scw = score_w.reshape(4, 128).T.astype(bf16)
    biasm = conv_b.reshape(4, 128).T.astype(np.float32)
    idsb = input_ids.astype(np.float32).astype(bf16)

    in_maps = [{"ids": np.ascontiguousarray(idsb[c:c + 1]), "gws": gws, "iot": iot,
                "scw": scw, "bias": biasm} for c in range(B)]
    res = run_bass_kernel_spmd(nc, in_maps, core_ids=list(range(B)), trace=TRACE)
    LAST_RESULT = res
    return np.stack([np.asarray(res.results[c]["out"]).astype(np.float32)
                     for c in range(B)])
